# revision 1
# baseline (speedup 1.0000x reference)
"""DeepseekV4-style attention (partial-RoPE LoRA-Q GQA sliding-window) on 8
Trainium2 NeuronCores.

Sharding: core c = 4*b + g handles batch b (of 2) and GQA group g (of 4):
q heads 4g..4g+3, kv head g, the matching column slices of Wqb/Wk/Wv and row
slice of Wo.  Each core computes a partial output `hidden[b]-attention @
Wo[g-slice]`; the host sums the four partials per batch.

All matmuls run in float32r (full PE rate at free-dim >= 256, ~1e-4 rel err).
Layout is "T-layout": Q^T/K^T stored [head_dim, seq] so QK^T and PV need no
transposes; only V needs 16 PE transposes back to natural layout.  Sliding
window + causal masking is applied with gpsimd.affine_select on the exp'd
tiles; the softmax denominator comes from an all-ones matmul that directly
yields a partition-broadcast sum.
"""

import numpy as np
import concourse.bass as bass
import concourse.mybir as mybir
import concourse.tile as tile
from concourse.bass_utils import run_bass_kernel_spmd

F32 = mybir.dt.float32
F32R = mybir.dt.float32r
ACTF = mybir.ActivationFunctionType
ALU = mybir.AluOpType

B, S, D = 2, 2048, 2048
H, KVH, HD = 16, 4, 128
ROT, LORA, WINDOW = 64, 512, 1024
ROPE_BASE = 10000.0
SCALE = HD ** -0.5

HPC = H // KVH          # 4 q heads per core
SB = 512                # free-dim block for matmuls
NSB = S // SB           # 4 seq blocks
KT = D // 128           # 16 contraction tiles over D
ST = S // 128           # 16 seq 128-chunks
N_CORES = 8


def _split_multiwaits(nc):
    """This image's walrus accepts only one embedded SyncWait per instruction;
    split Tile's multi-wait sync_infos into standalone event-semaphore waits."""
    n = 0
    for func in nc.m.functions:
        for bb in func.blocks:
            insts = list(bb.instructions)
            out = []
            changed = False
            for inst in insts:
                si = inst.sync_info
                if si is not None and si.on_wait and len(si.on_wait) > 1:
                    waits = list(si.on_wait)
                    for w in waits[:-1]:
                        ev = mybir.InstEventSemaphore(
                            name=f"{inst.name}_wsplit_{n}", ins=[], outs=[]
                        )
                        ev.engine = inst.engine
                        ev.sync_info = mybir.SyncInfo(on_wait=[w], on_update=[])
                        out.append(ev)
                        n += 1
                    inst.sync_info = mybir.SyncInfo(
                        on_wait=[waits[-1]], on_update=list(si.on_update or [])
                    )
                    changed = True
                out.append(inst)
            if changed:
                bb.instructions = out
    return n


def build_nc(debug=False):
    nc = bass.Bass()
    hid = nc.dram_tensor("hid", [D, S], F32R, kind="ExternalInput")
    wqa = nc.dram_tensor("wqa", [D, LORA], F32R, kind="ExternalInput")
    wqb = nc.dram_tensor("wqb", [LORA, HPC * HD], F32R, kind="ExternalInput")
    wkv = nc.dram_tensor("wkv", [D, 2 * HD], F32R, kind="ExternalInput")
    wo = nc.dram_tensor("wo", [HPC * HD, D], F32R, kind="ExternalInput")
    rcs = nc.dram_tensor("rcs", [128, S], F32R, kind="ExternalInput")
    out = nc.dram_tensor("out", [S, D], F32, kind="ExternalOutput")
    if debug:
        qt_dbg = nc.dram_tensor("qt_dbg", [128, HPC * S], F32R, kind="ExternalOutput")
        kt_dbg = nc.dram_tensor("kt_dbg", [128, S], F32R, kind="ExternalOutput")
        vn_dbg = nc.dram_tensor("vn_dbg", [128, S], F32R, kind="ExternalOutput")
        at_dbg = nc.dram_tensor("at_dbg", [128, HPC * S], F32R, kind="ExternalOutput")
    hidT = hid  # host supplies hidden[b] pre-transposed: [D, S], s contiguous

    with tile.TileContext(nc) as tc:
        with (
            tc.tile_pool(name="cst", bufs=1) as cst,
            tc.tile_pool(name="big", bufs=1) as big,
        ):
            # ---- constants ----
            ropeCC = cst.tile([64, S], F32R, tag="ropeCC")
            nc.sync.dma_start(out=ropeCC[:], in_=rcs[0:64, :])
            ropeSS = cst.tile([64, S], F32R, tag="ropeSS")
            nc.sync.dma_start(out=ropeSS[:], in_=rcs[64:128, :])
            onesf = cst.tile([128, 128], F32, tag="onesf")
            nc.vector.memset(onesf[:], 1.0)
            ones = cst.tile([128, 128], F32R, tag="ones")
            nc.vector.tensor_copy(ones[:], onesf[:])
            identf = cst.tile([128, 128], F32, tag="identf")
            nc.gpsimd.affine_select(
                out=identf[:], in_=onesf[:], pattern=[[1, 128]],
                compare_op=ALU.is_equal, fill=0.0, base=0, channel_multiplier=-1,
            )
            ident = cst.tile([128, 128], F32R, tag="ident")
            nc.vector.tensor_copy(ident[:], identf[:])

            # ---- persistent activations ----
            qT = big.tile([128, HPC * S], F32R, tag="qT")    # per-head Q^T [hd, s]
            kT = big.tile([128, S], F32R, tag="kT")
            vT = big.tile([128, S], F32R, tag="vT")
            vnat = big.tile([128, S], F32R, tag="vnat")      # V rows, 128-chunk t at cols t*128

            def rope_apply(dst, sl, rsl, rp):
                # dst rows 0:64 hold [x1; x2]; rotate in place (T-layout).
                # DVE ops need equal SBUF base partitions, so the half-swap
                # goes through a small SBUF->SBUF DMA.
                swp = rp.tile([64, SB], F32R, tag="swp")
                nc.sync.dma_start(out=swp[0:32, :], in_=dst[32:64, sl])
                nc.sync.dma_start(out=swp[32:64, :], in_=dst[0:32, sl])
                csb = rp.tile([64, SB], F32R, tag="csb")
                nc.vector.tensor_mul(csb[:], dst[0:64, sl], ropeCC[:, rsl])
                tsin = rp.tile([64, SB], F32R, tag="tsin")
                nc.vector.tensor_mul(tsin[:], swp[:], ropeSS[:, rsl])
                nc.vector.tensor_sub(dst[0:32, sl], csb[0:32, :], tsin[0:32, :])
                nc.vector.tensor_add(dst[32:64, sl], csb[32:64, :], tsin[32:64, :])

            with (
                tc.tile_pool(name="tmpA", bufs=1) as tmpA,
                tc.tile_pool(name="hp", bufs=4) as hp,
                tc.tile_pool(name="rp", bufs=2) as rp,
                tc.tile_pool(name="psA", bufs=1, space="PSUM") as psA,
                tc.tile_pool(name="psT", bufs=1, space="PSUM") as psT,
                tc.tile_pool(name="psB", bufs=1, space="PSUM") as psB,
            ):
                # ---- weights for stage 1/2 ----
                wqa_sb = tmpA.tile([128, KT * LORA], F32R, tag="wqa_sb")
                for k in range(KT):
                    nc.sync.dma_start(
                        out=wqa_sb[:, k * LORA:(k + 1) * LORA],
                        in_=wqa[k * 128:(k + 1) * 128, :],
                    )
                wkv_sb = tmpA.tile([128, KT * 256], F32R, tag="wkv_sb")
                for k in range(KT):
                    nc.sync.dma_start(
                        out=wkv_sb[:, k * 256:(k + 1) * 256],
                        in_=wkv[k * 128:(k + 1) * 128, :],
                    )
                wqb_sb = tmpA.tile([128, 4 * HPC * HD], F32R, tag="wqb_sb")
                for k in range(4):
                    nc.sync.dma_start(
                        out=wqb_sb[:, k * 512:(k + 1) * 512],
                        in_=wqb[k * 128:(k + 1) * 128, :],
                    )
                qaT = tmpA.tile([128, 4 * S], F32R, tag="qaT")  # qa^T, m-tile m at cols m*S

                # ---- stage 1: qa^T, k^T, v^T from hidden^T ----
                for sb_i in range(NSB):
                    sl = slice(sb_i * SB, (sb_i + 1) * SB)
                    pq = [
                        psA.tile([128, SB], F32, tag=f"pq{m}", name=f"pq{m}_{sb_i}")
                        for m in range(4)
                    ]
                    pk = psA.tile([128, SB], F32, tag="pk")
                    pv = psA.tile([128, SB], F32, tag="pv")
                    for k in range(KT):
                        ht = hp.tile([128, SB], F32R, tag="ht")
                        nc.sync.dma_start(
                            out=ht[:], in_=hidT[k * 128:(k + 1) * 128, sl]
                        )
                        st, sp = (k == 0), (k == KT - 1)
                        for m in range(4):
                            nc.tensor.matmul(
                                pq[m][:],
                                wqa_sb[:, k * LORA + m * 128: k * LORA + (m + 1) * 128],
                                ht[:], start=st, stop=sp,
                            )
                        nc.tensor.matmul(
                            pk[:], wkv_sb[:, k * 256: k * 256 + 128], ht[:],
                            start=st, stop=sp,
                        )
                        nc.tensor.matmul(
                            pv[:], wkv_sb[:, k * 256 + 128: k * 256 + 256], ht[:],
                            start=st, stop=sp,
                        )
                    for m in range(4):
                        nc.scalar.copy(qaT[:, m * S + sb_i * SB: m * S + (sb_i + 1) * SB],
                                       pq[m][:])
                    nc.scalar.copy(kT[:, sl], pk[:])
                    nc.scalar.copy(vT[:, sl], pv[:])
                    rope_apply(kT, sl, sl, rp)
                    # V natural: PE-transpose the 4 128-chunks of this block
                    for t in range(sb_i * 4, sb_i * 4 + 4):
                        tp = psT.tile([128, 128], F32R, tag="tp")
                        nc.tensor.transpose(tp[:], vT[:, t * 128:(t + 1) * 128], ident[:])
                        nc.vector.tensor_copy(vnat[:, t * 128:(t + 1) * 128], tp[:])

                # ---- stage 2: q^T per head ----
                for sb_i in range(NSB):
                    sl = slice(sb_i * SB, (sb_i + 1) * SB)
                    for h in range(HPC):
                        p2 = psB.tile([128, SB], F32, tag="p2")
                        for k in range(4):
                            nc.tensor.matmul(
                                p2[:],
                                wqb_sb[:, k * 512 + h * 128: k * 512 + (h + 1) * 128],
                                qaT[:, k * S + sb_i * SB: k * S + (sb_i + 1) * SB],
                                start=(k == 0), stop=(k == 3),
                            )
                        nc.scalar.copy(qT[:, h * S + sb_i * SB: h * S + (sb_i + 1) * SB],
                                       p2[:])
                        rope_apply(qT, slice(h * S + sb_i * SB, h * S + (sb_i + 1) * SB),
                                   sl, rp)

            if debug:
                nc.sync.dma_start(out=qt_dbg[:], in_=qT[:])
                nc.sync.dma_start(out=kt_dbg[:], in_=kT[:])
                nc.sync.dma_start(out=vn_dbg[:], in_=vnat[:])

            # ---- stage 3: attention ----
            with tc.tile_pool(name="bigB", bufs=1) as bigB:
                attnT = bigB.tile([128, HPC * S], F32R, tag="attnT")
                with (
                    tc.tile_pool(name="ex", bufs=4) as ex,
                    tc.tile_pool(name="rc", bufs=2) as rc,
                    tc.tile_pool(name="psL", bufs=2, space="PSUM") as psL,
                    tc.tile_pool(name="psO", bufs=2, space="PSUM") as psO,
                    tc.tile_pool(name="psD", bufs=2, space="PSUM") as psD,
                ):
                    for h in range(HPC):
                        for qb in range(NSB):
                            q0 = qb * SB
                            qsl = slice(h * S + q0, h * S + q0 + SB)
                            kt_lo = max(0, q0 - WINDOW + 1) // 128
                            kt_hi = q0 // 128 + 3
                            po = psO.tile([128, SB], F32, tag="po")
                            pd = psD.tile([128, SB], F32, tag="pd")
                            for kt in range(kt_lo, kt_hi + 1):
                                dp = kt * 128 - q0
                                pl = psL.tile([128, SB], F32, tag="pl")
                                nc.tensor.matmul(
                                    pl[:], kT[:, kt * 128:(kt + 1) * 128], qT[:, qsl],
                                    start=True, stop=True,
                                )
                                e = ex.tile([128, SB], F32R, tag="e")
                                nc.scalar.activation(e[:], pl[:], ACTF.Exp, scale=SCALE)
                                if dp >= 0:
                                    # causal edge: keep j - i - dp >= 0
                                    nc.gpsimd.affine_select(
                                        out=e[:], in_=e[:], pattern=[[1, SB]],
                                        compare_op=ALU.is_ge, fill=0.0,
                                        base=-dp, channel_multiplier=-1,
                                    )
                                elif dp <= SB - WINDOW:
                                    # window edge: keep (q0+j)-(k0+i) = j-i-dp
                                    # < WINDOW, i.e. WINDOW-1+dp + i - j >= 0
                                    nc.gpsimd.affine_select(
                                        out=e[:], in_=e[:], pattern=[[-1, SB]],
                                        compare_op=ALU.is_ge, fill=0.0,
                                        base=WINDOW - 1 + dp, channel_multiplier=1,
                                    )
                                st, sp = (kt == kt_lo), (kt == kt_hi)
                                nc.tensor.matmul(
                                    po[:], vnat[:, kt * 128:(kt + 1) * 128], e[:],
                                    start=st, stop=sp,
                                )
                                nc.tensor.matmul(pd[:], ones[:], e[:], start=st, stop=sp)
                            rec = rc.tile([128, SB], F32, tag="rec")
                            nc.vector.reciprocal(rec[:], pd[:])
                            nc.vector.tensor_mul(attnT[:, qsl], po[:], rec[:])

                if debug:
                    nc.sync.dma_start(out=at_dbg[:], in_=attnT[:])

                # ---- stage 4: output projection (partial over this head group) ----
                with (
                    tc.tile_pool(name="tmpB", bufs=1) as tmpB,
                    tc.tile_pool(name="od", bufs=2) as od,
                    tc.tile_pool(name="psW", bufs=4, space="PSUM") as psW,
                ):
                    wo_sb = tmpB.tile([128, HPC * D], F32R, tag="wo_sb")
                    for h in range(HPC):
                        nc.sync.dma_start(
                            out=wo_sb[:, h * D:(h + 1) * D],
                            in_=wo[h * 128:(h + 1) * 128, :],
                        )
                    for t in range(ST):
                        ot = od.tile([128, D], F32, tag="ot")
                        for n in range(4):
                            pw = psW.tile([128, SB], F32, tag="pw")
                            for h in range(HPC):
                                nc.tensor.matmul(
                                    pw[:],
                                    attnT[:, h * S + t * 128: h * S + (t + 1) * 128],
                                    wo_sb[:, h * D + n * SB: h * D + (n + 1) * SB],
                                    start=(h == 0), stop=(h == HPC - 1),
                                )
                            nc.scalar.copy(ot[:, n * SB:(n + 1) * SB], pw[:])
                        nc.sync.dma_start(
                            out=out[t * 128:(t + 1) * 128, :], in_=ot[:]
                        )
    _split_multiwaits(nc)
    return nc


_NC = None


def _get_nc():
    global _NC
    if _NC is None:
        _NC = build_nc()
    return _NC


def _make_in_maps(hidden, position_ids, Wqa, Wqb, Wk, Wv, Wo):
    hidden = np.asarray(hidden, dtype=np.float32)
    position_ids = np.asarray(position_ids)
    Wqa = np.ascontiguousarray(np.asarray(Wqa, dtype=np.float32))
    Wqb = np.asarray(Wqb, dtype=np.float32)
    Wk = np.asarray(Wk, dtype=np.float32)
    Wv = np.asarray(Wv, dtype=np.float32)
    Wo = np.asarray(Wo, dtype=np.float32)

    inv_freq = 1.0 / (ROPE_BASE ** (np.arange(0, ROT, 2, dtype=np.float32) / ROT))
    in_maps = []
    for c in range(N_CORES):
        b, g = c // KVH, c % KVH
        pos = position_ids[b].astype(np.float32)
        freqs = pos[:, None] * inv_freq[None, :]        # [S, 32]
        cosT = np.cos(freqs).T.astype(np.float32)       # [32, S]
        sinT = np.sin(freqs).T.astype(np.float32)
        rcs = np.concatenate([cosT, cosT, sinT, sinT], axis=0)  # [128, S]
        in_maps.append({
            "hid": np.ascontiguousarray(hidden[b].T),
            "wqa": Wqa,
            "wqb": np.ascontiguousarray(Wqb[:, g * HPC * HD:(g + 1) * HPC * HD]),
            "wkv": np.ascontiguousarray(
                np.concatenate(
                    [Wk[:, g * HD:(g + 1) * HD], Wv[:, g * HD:(g + 1) * HD]], axis=1
                )
            ),
            "wo": np.ascontiguousarray(Wo[g * HPC * HD:(g + 1) * HPC * HD, :]),
            "rcs": np.ascontiguousarray(rcs),
        })
    return in_maps


def _run(inputs, trace=False):
    nc = _get_nc()
    in_maps = _make_in_maps(**inputs)
    res = run_bass_kernel_spmd(nc, in_maps, list(range(N_CORES)), trace=trace)
    out = np.zeros((B, S, D), dtype=np.float32)
    for c in range(N_CORES):
        out[c // KVH] += res.results[c]["out"]
    return out, res


def kernel(**inputs) -> np.ndarray:
    return _run(inputs, trace=False)[0]



# revision 4
# speedup vs baseline: 1.1328x; 1.1328x over previous
"""DeepseekV4-style attention (partial-RoPE LoRA-Q GQA sliding-window) on 8
Trainium2 NeuronCores.

Sharding: core c = 4*b + g handles batch b (of 2) and GQA group g (of 4):
q heads 4g..4g+3, kv head g, the matching column slices of Wq_eff/Wk/Wv and
row slice of Wo.  Each core computes a partial output; the host sums the four
partials per batch.

vs the previous version:
- The LoRA Q projection is folded on the host (W_eff = Wqa @ Wqb slice), so
  stage 2 disappears and q^T comes straight out of stage 1.
- DMA emission order feeds the PE within ~2us (k-interleaved weight+hidden
  tiles for block 0) and prefetches Wo during stage 1.
- RoPE's half-swap uses a signed 64x64 permutation matmul on the PE instead
  of two SBUF->SBUF DMAs.
- Attention tiles are narrowed at the causal/window edges (N clamped to
  >=256 to stay at full fp32r rate).
- softmax uses reciprocal_approx_fast (5x cheaper than reciprocal).
- Output projection is interleaved per q-block with attention so the PE
  never drains between stages; PSUM->SBUF evacuations are split between
  scalar and vector engines.
"""

import numpy as np
import concourse.bass as bass
import concourse.mybir as mybir
import concourse.tile as tile
from concourse.bass_utils import run_bass_kernel_spmd

F32 = mybir.dt.float32
F32R = mybir.dt.float32r
ACTF = mybir.ActivationFunctionType
ALU = mybir.AluOpType

B, S, D = 2, 2048, 2048
H, KVH, HD = 16, 4, 128
ROT, LORA, WINDOW = 64, 512, 1024
ROPE_BASE = 10000.0
SCALE = HD ** -0.5

HPC = H // KVH          # 4 q heads per core
SB = 512                # free-dim block for matmuls
NSB = S // SB           # 4 seq blocks
KT = D // 128           # 16 contraction tiles over D
ST = S // 128           # 16 seq 128-chunks
N_CORES = 8


def _split_multiwaits(nc):
    """This image's walrus accepts only one embedded SyncWait per instruction;
    split Tile's multi-wait sync_infos into standalone event-semaphore waits."""
    n = 0
    for func in nc.m.functions:
        for bb in func.blocks:
            insts = list(bb.instructions)
            out = []
            changed = False
            for inst in insts:
                si = inst.sync_info
                if si is not None and si.on_wait and len(si.on_wait) > 1:
                    waits = list(si.on_wait)
                    for w in waits[:-1]:
                        ev = mybir.InstEventSemaphore(
                            name=f"{inst.name}_wsplit_{n}", ins=[], outs=[]
                        )
                        ev.engine = inst.engine
                        ev.sync_info = mybir.SyncInfo(on_wait=[w], on_update=[])
                        out.append(ev)
                        n += 1
                    inst.sync_info = mybir.SyncInfo(
                        on_wait=[waits[-1]], on_update=list(si.on_update or [])
                    )
                    changed = True
                out.append(inst)
            if changed:
                bb.instructions = out
    return n


def build_nc():
    nc = bass.Bass()
    hid = nc.dram_tensor("hid", [D, S], F32R, kind="ExternalInput")
    weff = nc.dram_tensor("weff", [D, HPC * HD], F32R, kind="ExternalInput")
    wkv = nc.dram_tensor("wkv", [D, 2 * HD], F32R, kind="ExternalInput")
    wo = nc.dram_tensor("wo", [HPC * HD, D], F32R, kind="ExternalInput")
    rcs = nc.dram_tensor("rcs", [128, S], F32R, kind="ExternalInput")
    out = nc.dram_tensor("out", [S, D], F32, kind="ExternalOutput")

    with tile.TileContext(nc) as tc:
        with (
            tc.tile_pool(name="cst", bufs=1) as cst,
            tc.tile_pool(name="big", bufs=1) as big,
        ):
            # ---- small constants (engine-built, no DMA) ----
            onesf = cst.tile([128, 128], F32, tag="onesf")
            nc.vector.memset(onesf[:], 1.0)
            ones = cst.tile([128, 128], F32R, tag="ones")
            nc.vector.tensor_copy(ones[:], onesf[:])
            identf = cst.tile([128, 128], F32, tag="identf")
            nc.gpsimd.affine_select(
                out=identf[:], in_=onesf[:], pattern=[[1, 128]],
                compare_op=ALU.is_equal, fill=0.0, base=0, channel_multiplier=-1,
            )
            ident = cst.tile([128, 128], F32R, tag="ident")
            nc.vector.tensor_copy(ident[:], identf[:])
            # signed rope permutation P: P[32+i, i] = -1, P[j, 32+j] = +1
            negf = cst.tile([64, 32], F32, tag="negf")
            nc.vector.memset(negf[:], -1.0)
            posf = cst.tile([64, 32], F32, tag="posf")
            nc.vector.memset(posf[:], 1.0)
            permf = cst.tile([64, 64], F32, tag="permf")
            # cols 0:32: keep p - f - 32 == 0 of (-1)
            nc.gpsimd.affine_select(
                out=permf[:, 0:32], in_=negf[:], pattern=[[-1, 32]],
                compare_op=ALU.is_equal, fill=0.0, base=-32, channel_multiplier=1,
            )
            # cols 32:64: keep p - f == 0 of (+1)
            nc.gpsimd.affine_select(
                out=permf[:, 32:64], in_=posf[:], pattern=[[-1, 32]],
                compare_op=ALU.is_equal, fill=0.0, base=0, channel_multiplier=1,
            )
            perm = cst.tile([64, 64], F32R, tag="perm")
            nc.vector.tensor_copy(perm[:], permf[:])

            # ---- persistent activations ----
            qT = big.tile([128, HPC * S], F32R, tag="qT")    # per-head Q^T [hd, s]
            kT = big.tile([128, S], F32R, tag="kT")
            vnat = big.tile([128, S], F32R, tag="vnat")      # V rows, chunk t at cols t*128
            ropeCC = big.tile([64, S], F32R, tag="ropeCC")
            ropeSS = big.tile([64, S], F32R, tag="ropeSS")
            wo_sb = big.tile([128, HPC * D], F32R, tag="wo_sb")

            with (
                tc.tile_pool(name="wp", bufs=1) as wp,
                tc.tile_pool(name="h0p", bufs=1) as h0p,
                tc.tile_pool(name="hp", bufs=8) as hp,
                tc.tile_pool(name="vt", bufs=2) as vt,
                tc.tile_pool(name="rp", bufs=2) as rp,
                tc.tile_pool(name="psA", bufs=1, space="PSUM") as psA,
                tc.tile_pool(name="psS", bufs=1, space="PSUM") as psS,
            ):
                # ---- stage-1 weights + block-0 hidden, k-interleaved ----
                weff_sb = wp.tile([128, KT * 512], F32R, tag="weff_sb")
                wkv_sb = wp.tile([128, KT * 256], F32R, tag="wkv_sb")
                ht0 = []
                for k in range(KT):
                    nc.sync.dma_start(
                        out=weff_sb[:, k * 512:(k + 1) * 512],
                        in_=weff[k * 128:(k + 1) * 128, :],
                    )
                    nc.sync.dma_start(
                        out=wkv_sb[:, k * 256:(k + 1) * 256],
                        in_=wkv[k * 128:(k + 1) * 128, :],
                    )
                    t = h0p.tile([128, SB], F32R, tag=f"h0_{k}")
                    nc.sync.dma_start(out=t[:], in_=hid[k * 128:(k + 1) * 128, 0:SB])
                    ht0.append(t)
                # rope tables (needed from ~20us in)
                nc.sync.dma_start(out=ropeCC[:], in_=rcs[0:64, :])
                nc.sync.dma_start(out=ropeSS[:], in_=rcs[64:128, :])

                def rope_apply(dst, sl, rsl):
                    # dst rows 0:64 hold [x1; x2]; out = x*cos + P^T(x*sin)
                    tsin = rp.tile([64, SB], F32R, tag="tsin")
                    nc.vector.tensor_mul(tsin[:], dst[0:64, sl], ropeSS[:, rsl])
                    csb = rp.tile([64, SB], F32R, tag="csb")
                    nc.vector.tensor_mul(csb[:], dst[0:64, sl], ropeCC[:, rsl])
                    pP = psS.tile([64, SB], F32, tag="pP")
                    nc.tensor.matmul(pP[:], perm[:], tsin[:], start=True, stop=True)
                    nc.vector.tensor_add(dst[0:64, sl], csb[:], pP[:])

                # ---- stage 1: q^T (folded), k^T, v -> vnat ----
                for sb_i in range(NSB):
                    sl = slice(sb_i * SB, (sb_i + 1) * SB)
                    pq = [
                        psA.tile([128, SB], F32, tag=f"pq{m}", name=f"pq{m}_{sb_i}")
                        for m in range(HPC)
                    ]
                    pk = psA.tile([128, SB], F32, tag="pk")
                    pv = psA.tile([128, SB], F32, tag="pv")
                    for k in range(KT):
                        if sb_i == 0:
                            ht = ht0[k]
                        else:
                            ht = hp.tile([128, SB], F32R, tag="ht")
                            nc.sync.dma_start(
                                out=ht[:], in_=hid[k * 128:(k + 1) * 128, sl]
                            )
                        st, sp = (k == 0), (k == KT - 1)
                        for m in range(HPC):
                            nc.tensor.matmul(
                                pq[m][:],
                                weff_sb[:, k * 512 + m * 128: k * 512 + (m + 1) * 128],
                                ht[:], start=st, stop=sp,
                            )
                        nc.tensor.matmul(
                            pk[:], wkv_sb[:, k * 256: k * 256 + 128], ht[:],
                            start=st, stop=sp,
                        )
                        nc.tensor.matmul(
                            pv[:], wkv_sb[:, k * 256 + 128: k * 256 + 256], ht[:],
                            start=st, stop=sp,
                        )
                    for m in range(HPC):
                        qsl = slice(m * S + sb_i * SB, m * S + (sb_i + 1) * SB)
                        nc.scalar.copy(qT[:, qsl], pq[m][:])
                        rope_apply(qT, qsl, sl)
                    nc.scalar.copy(kT[:, sl], pk[:])
                    rope_apply(kT, sl, sl)
                    # V natural: PE-transpose the 4 128-chunks of this block
                    vtmp = vt.tile([128, SB], F32R, tag="vtmp")
                    nc.scalar.copy(vtmp[:], pv[:])
                    for tt in range(4):
                        tp = psS.tile([128, 128], F32R, tag="tp")
                        nc.tensor.transpose(
                            tp[:], vtmp[:, tt * 128:(tt + 1) * 128], ident[:]
                        )
                        nc.vector.tensor_copy(
                            vnat[:, (sb_i * 4 + tt) * 128:(sb_i * 4 + tt + 1) * 128],
                            tp[:],
                        )

            # ---- Wo prefetch (lands well before stage 4 needs it) ----
            for h in range(HPC):
                nc.sync.dma_start(
                    out=wo_sb[:, h * D:(h + 1) * D],
                    in_=wo[h * 128:(h + 1) * 128, :],
                )

            # ---- stage 3+4: attention with interleaved output projection ----
            with (
                tc.tile_pool(name="at", bufs=2) as at,
                tc.tile_pool(name="ex", bufs=4) as ex,
                tc.tile_pool(name="rc", bufs=2) as rc,
                tc.tile_pool(name="ob", bufs=4) as ob,
                tc.tile_pool(name="psL", bufs=2, space="PSUM") as psL,
                tc.tile_pool(name="psO", bufs=2, space="PSUM") as psO,
                tc.tile_pool(name="psD", bufs=2, space="PSUM") as psD,
                tc.tile_pool(name="psW", bufs=2, space="PSUM") as psW,
            ):
                for qb in range(NSB):
                    q0 = qb * SB
                    kt_lo = max(0, q0 - WINDOW + 1) // 128
                    kt_hi = q0 // 128 + 3
                    attnT = at.tile([128, HPC * SB], F32R, tag="attnT",
                                    name=f"attnT_{qb}")
                    for h in range(HPC):
                        qsl = slice(h * S + q0, h * S + q0 + SB)
                        po = psO.tile([128, SB], F32, tag="po")
                        pd = psD.tile([128, SB], F32, tag="pd")
                        for kt in range(kt_lo, kt_hi + 1):
                            dp = kt * 128 - q0
                            # valid cols [lo, hi), clamped to N >= 256 (full
                            # fp32r rate) and even N (s3d3 fp32r ISA rule)
                            lo = min(max(0, dp), SB - 256)
                            hi_i = dp + WINDOW + 127
                            hi = max(min(SB, hi_i + (hi_i & 1)), 256)
                            n = hi - lo
                            ksl = slice(kt * 128, (kt + 1) * 128)
                            pl = psL.tile([128, SB], F32, tag="pl")
                            nc.tensor.matmul(
                                pl[:, lo:hi], kT[:, ksl],
                                qT[:, h * S + q0 + lo: h * S + q0 + hi],
                                start=True, stop=True,
                            )
                            e = ex.tile([128, SB], F32R, tag="e")
                            nc.scalar.activation(
                                e[:, lo:hi], pl[:, lo:hi], ACTF.Exp, scale=SCALE
                            )
                            if dp >= 0:
                                # causal: keep j - i - dp >= 0 (local f = j - lo)
                                nc.gpsimd.affine_select(
                                    out=e[:, lo:hi], in_=e[:, lo:hi],
                                    pattern=[[1, n]], compare_op=ALU.is_ge,
                                    fill=0.0, base=lo - dp, channel_multiplier=-1,
                                )
                            elif dp <= -(WINDOW - SB + 1):
                                # window: keep i - j + dp + W-1 >= 0 (lo == 0)
                                nc.gpsimd.affine_select(
                                    out=e[:, lo:hi], in_=e[:, lo:hi],
                                    pattern=[[-1, n]], compare_op=ALU.is_ge,
                                    fill=0.0, base=WINDOW - 1 + dp,
                                    channel_multiplier=1,
                                )
                            st, sp = (kt == kt_lo), (kt == kt_hi)
                            nc.tensor.matmul(
                                po[:, lo:hi], vnat[:, ksl], e[:, lo:hi],
                                start=st, stop=sp,
                            )
                            nc.tensor.matmul(
                                pd[:, lo:hi], ones[:], e[:, lo:hi],
                                start=st, stop=sp,
                            )
                        rec = rc.tile([128, SB], F32, tag="rec")
                        nc.vector.reciprocal(rec[:], pd[:])
                        nc.vector.tensor_mul(
                            attnT[:, h * SB:(h + 1) * SB], po[:], rec[:]
                        )
                    # stage 4 for this q-block: out[q, :] partial = attn @ Wo
                    for tl in range(4):
                        t = qb * 4 + tl
                        for n4 in range(4):
                            pw = psW.tile([128, SB], F32, tag="pw")
                            for h in range(HPC):
                                nc.tensor.matmul(
                                    pw[:],
                                    attnT[:, h * SB + tl * 128: h * SB + (tl + 1) * 128],
                                    wo_sb[:, h * D + n4 * SB: h * D + (n4 + 1) * SB],
                                    start=(h == 0), stop=(h == HPC - 1),
                                )
                            obuf = ob.tile([128, SB], F32, tag="obuf")
                            if n4 % 2 == 0:
                                nc.scalar.copy(obuf[:], pw[:])
                            else:
                                nc.vector.tensor_copy(obuf[:], pw[:])
                            nc.sync.dma_start(
                                out=out[t * 128:(t + 1) * 128,
                                        n4 * SB:(n4 + 1) * SB],
                                in_=obuf[:],
                            )
    _split_multiwaits(nc)
    return nc


_NC = None


def _get_nc():
    global _NC
    if _NC is None:
        _NC = build_nc()
    return _NC


def _make_in_maps(hidden, position_ids, Wqa, Wqb, Wk, Wv, Wo):
    hidden = np.asarray(hidden, dtype=np.float32)
    position_ids = np.asarray(position_ids)
    Wqa = np.asarray(Wqa, dtype=np.float32)
    Wqb = np.asarray(Wqb, dtype=np.float32)
    Wk = np.asarray(Wk, dtype=np.float32)
    Wv = np.asarray(Wv, dtype=np.float32)
    Wo = np.asarray(Wo, dtype=np.float32)
    weff_full = Wqa @ Wqb  # [D, H*HD]; exact assoc. fold of the LoRA Q proj

    inv_freq = 1.0 / (ROPE_BASE ** (np.arange(0, ROT, 2, dtype=np.float32) / ROT))
    in_maps = []
    for c in range(N_CORES):
        b, g = c // KVH, c % KVH
        pos = position_ids[b].astype(np.float32)
        freqs = pos[:, None] * inv_freq[None, :]        # [S, 32]
        cosT = np.cos(freqs).T.astype(np.float32)       # [32, S]
        sinT = np.sin(freqs).T.astype(np.float32)
        rcs = np.concatenate([cosT, cosT, sinT, sinT], axis=0)  # [128, S]
        in_maps.append({
            "hid": np.ascontiguousarray(hidden[b].T),
            "weff": np.ascontiguousarray(
                weff_full[:, g * HPC * HD:(g + 1) * HPC * HD]
            ),
            "wkv": np.ascontiguousarray(
                np.concatenate(
                    [Wk[:, g * HD:(g + 1) * HD], Wv[:, g * HD:(g + 1) * HD]], axis=1
                )
            ),
            "wo": np.ascontiguousarray(Wo[g * HPC * HD:(g + 1) * HPC * HD, :]),
            "rcs": np.ascontiguousarray(rcs),
        })
    return in_maps


def _run(inputs, trace=False):
    nc = _get_nc()
    in_maps = _make_in_maps(**inputs)
    res = run_bass_kernel_spmd(nc, in_maps, list(range(N_CORES)), trace=trace)
    out = np.zeros((B, S, D), dtype=np.float32)
    for c in range(N_CORES):
        out[c // KVH] += res.results[c]["out"]
    return out, res


def kernel(**inputs) -> np.ndarray:
    return _run(inputs, trace=False)[0]


# revision 10
# speedup vs baseline: 1.1561x; 1.0206x over previous
"""DeepseekV4-style attention (partial-RoPE LoRA-Q GQA sliding-window) on 8
Trainium2 NeuronCores.

Sharding: core c = 4*b + g handles batch b (of 2) and GQA group g (of 4):
q heads 4g..4g+3, kv head g, the matching column slices of Wq_eff/Wk/Wv and
row slice of Wo.  Each core computes a partial output; the host sums the four
partials per batch.

Design notes:
- LoRA Q projection folded on the host (W_eff = Wqa @ Wqb slice).
- Host packs hidden/weights into the exact SBUF layouts so each tensor needs
  a handful of large DMAs; startup is finely interleaved so the first matmul
  fires ~2us in.
- RoPE's half-swap is a signed 64x64 permutation matmul on the PE; rope and
  V-transpose emissions for block b are deferred into block b+1's k-loop so
  the PE FIFO is never blocked by a DVE-dependent instruction.
- Attention tiles narrowed at causal/window edges (N even, >=256 to satisfy
  s3d3 fp32r ISA rules at full rate); chunk loop is software-pipelined
  (QK of chunk c+2 is emitted before PV/pd of chunk c).
- Output projection for q-block qb-1 is emitted interleaved after each head
  of q-block qb, so stage-4 matmuls fill the PE while DVE finishes the
  softmax normalization of qb.
"""

import numpy as np
import concourse.bass as bass
import concourse.mybir as mybir
import concourse.tile as tile
from concourse.bass_utils import run_bass_kernel_spmd

F32 = mybir.dt.float32
F32R = mybir.dt.float32r
ACTF = mybir.ActivationFunctionType
ALU = mybir.AluOpType

B, S, D = 2, 2048, 2048
H, KVH, HD = 16, 4, 128
ROT, LORA, WINDOW = 64, 512, 1024
ROPE_BASE = 10000.0
SCALE = HD ** -0.5

HPC = H // KVH          # 4 q heads per core
SB = 512                # free-dim block for matmuls
NSB = S // SB           # 4 seq blocks
KT = D // 128           # 16 contraction tiles over D
ST = S // 128           # 16 seq 128-chunks
N_CORES = 8
PIPE = 2                # attention chunk software-pipeline depth


def _split_multiwaits(nc):
    """This image's walrus accepts only one embedded SyncWait per instruction;
    split Tile's multi-wait sync_infos into standalone event-semaphore waits."""
    n = 0
    for func in nc.m.functions:
        for bb in func.blocks:
            insts = list(bb.instructions)
            out = []
            changed = False
            for inst in insts:
                si = inst.sync_info
                if si is not None and si.on_wait and len(si.on_wait) > 1:
                    waits = list(si.on_wait)
                    for w in waits[:-1]:
                        ev = mybir.InstEventSemaphore(
                            name=f"{inst.name}_wsplit_{n}", ins=[], outs=[]
                        )
                        ev.engine = inst.engine
                        ev.sync_info = mybir.SyncInfo(on_wait=[w], on_update=[])
                        out.append(ev)
                        n += 1
                    inst.sync_info = mybir.SyncInfo(
                        on_wait=[waits[-1]], on_update=list(si.on_update or [])
                    )
                    changed = True
                out.append(inst)
            if changed:
                bb.instructions = out
    return n


def build_nc():
    nc = bass.Bass()
    # host-packed layouts: hid col = blk*8192 + k*512 + c; weff col = k*512+c;
    # wkv col = k*256+c; wo col = h*2048+c
    hid = nc.dram_tensor("hid", [128, NSB * KT * SB], F32R, kind="ExternalInput")
    weff = nc.dram_tensor("weff", [128, KT * 512], F32R, kind="ExternalInput")
    wkv = nc.dram_tensor("wkv", [128, KT * 256], F32R, kind="ExternalInput")
    wo = nc.dram_tensor("wo", [128, HPC * D], F32R, kind="ExternalInput")
    rcs = nc.dram_tensor("rcs", [128, S], F32R, kind="ExternalInput")
    out = nc.dram_tensor("out", [S, D], F32, kind="ExternalOutput")

    with tile.TileContext(nc) as tc:
        with (
            tc.tile_pool(name="cst", bufs=1) as cst,
            tc.tile_pool(name="big", bufs=1) as big,
        ):
            # ---- small constants (engine-built, no DMA) ----
            onesf = cst.tile([128, 128], F32, tag="onesf")
            nc.vector.memset(onesf[:], 1.0)
            ones = cst.tile([128, 128], F32R, tag="ones")
            nc.vector.tensor_copy(ones[:], onesf[:])
            identf = cst.tile([128, 128], F32, tag="identf")
            nc.gpsimd.affine_select(
                out=identf[:], in_=onesf[:], pattern=[[1, 128]],
                compare_op=ALU.is_equal, fill=0.0, base=0, channel_multiplier=-1,
            )
            ident = cst.tile([128, 128], F32R, tag="ident")
            nc.vector.tensor_copy(ident[:], identf[:])
            # signed rope permutation P: P[32+i, i] = -1, P[j, 32+j] = +1
            negf = cst.tile([64, 32], F32, tag="negf")
            nc.vector.memset(negf[:], -1.0)
            posf = cst.tile([64, 32], F32, tag="posf")
            nc.vector.memset(posf[:], 1.0)
            permf = cst.tile([64, 64], F32, tag="permf")
            nc.gpsimd.affine_select(
                out=permf[:, 0:32], in_=negf[:], pattern=[[-1, 32]],
                compare_op=ALU.is_equal, fill=0.0, base=-32, channel_multiplier=1,
            )
            nc.gpsimd.affine_select(
                out=permf[:, 32:64], in_=posf[:], pattern=[[-1, 32]],
                compare_op=ALU.is_equal, fill=0.0, base=0, channel_multiplier=1,
            )
            perm = cst.tile([64, 64], F32R, tag="perm")
            nc.vector.tensor_copy(perm[:], permf[:])

            # ---- persistent activations ----
            qT = big.tile([128, HPC * S], F32R, tag="qT")    # per-head Q^T [hd, s]
            kT = big.tile([128, S], F32R, tag="kT")
            vnat = big.tile([128, S], F32R, tag="vnat")      # V rows, chunk t at cols t*128
            wo_sb = big.tile([128, HPC * D], F32R, tag="wo_sb")

            with (
                tc.tile_pool(name="wp", bufs=1) as wp,
                tc.tile_pool(name="hp", bufs=5) as hp,
                tc.tile_pool(name="vt", bufs=2) as vt,
                tc.tile_pool(name="rp", bufs=2) as rp,
                tc.tile_pool(name="psA", bufs=1, space="PSUM") as psA,
                tc.tile_pool(name="psT", bufs=1, space="PSUM") as psT,
                tc.tile_pool(name="psP", bufs=1, space="PSUM") as psP,
            ):
                weff_sb = wp.tile([128, KT * 512], F32R, tag="weff_sb")
                wkv_sb = wp.tile([128, KT * 256], F32R, tag="wkv_sb")
                ropeCC = wp.tile([64, S], F32R, tag="ropeCC")
                ropeSS = wp.tile([64, S], F32R, tag="ropeSS")

                def dma_w(g0, g1):
                    nc.sync.dma_start(
                        out=weff_sb[:, g0 * 512:g1 * 512],
                        in_=weff[:, g0 * 512:g1 * 512],
                    )
                    nc.sync.dma_start(
                        out=wkv_sb[:, g0 * 256:g1 * 256],
                        in_=wkv[:, g0 * 256:g1 * 256],
                    )

                def dma_ht(sb_i, qd):
                    t = hp.tile([128, 2048], F32R, tag="ht", name=f"ht_{sb_i}_{qd}")
                    nc.sync.dma_start(
                        out=t[:],
                        in_=hid[:, sb_i * 8192 + qd * 2048:
                                sb_i * 8192 + (qd + 1) * 2048],
                    )
                    return t

                # startup interleave: weight k-groups racing block-0 hidden
                h0t = []
                dma_w(0, 1)
                h0t.append(dma_ht(0, 0))
                dma_w(1, 4)
                h0t.append(dma_ht(0, 1))
                dma_w(4, 8)
                h0t.append(dma_ht(0, 2))
                dma_w(8, 12)
                h0t.append(dma_ht(0, 3))
                dma_w(12, 16)
                nc.sync.dma_start(out=ropeCC[:], in_=rcs[0:64, :])
                nc.sync.dma_start(out=ropeSS[:], in_=rcs[64:128, :])

                def rope_emit(dst, csl, rsl):
                    # dst rows 0:64 hold [x1; x2]; out = x*cos + P^T(x*sin)
                    tsin = rp.tile([64, SB], F32R, tag="tsin")
                    nc.vector.tensor_mul(tsin[:], dst[0:64, csl], ropeSS[:, rsl])
                    csb = rp.tile([64, SB], F32R, tag="csb")
                    nc.vector.tensor_mul(csb[:], dst[0:64, csl], ropeCC[:, rsl])
                    pP = psP.tile([64, SB], F32, tag="pP")
                    nc.tensor.matmul(pP[:], perm[:], tsin[:], start=True, stop=True)
                    nc.vector.tensor_add(dst[0:64, csl], csb[:], pP[:])

                # ---- stage 1: q^T (folded), k^T, v -> vnat ----
                pending = []   # deferred rope/transpose emissions from prev block
                for sb_i in range(NSB):
                    sl = slice(sb_i * SB, (sb_i + 1) * SB)
                    if sb_i > 0:
                        hts = [dma_ht(sb_i, qd) for qd in range(4)]
                    else:
                        hts = h0t
                    pq = [
                        psA.tile([128, SB], F32, tag=f"pq{m}", name=f"pq{m}_{sb_i}")
                        for m in range(HPC)
                    ]
                    pk = psA.tile([128, SB], F32, tag="pk")
                    pv = psA.tile([128, SB], F32, tag="pv")
                    for k in range(KT):
                        hsl = hts[k // 4][:, (k % 4) * 512:(k % 4 + 1) * 512]
                        st, sp = (k == 0), (k == KT - 1)
                        for m in range(HPC):
                            nc.tensor.matmul(
                                pq[m][:],
                                weff_sb[:, k * 512 + m * 128: k * 512 + (m + 1) * 128],
                                hsl, start=st, stop=sp,
                            )
                        nc.tensor.matmul(
                            pk[:], wkv_sb[:, k * 256: k * 256 + 128], hsl,
                            start=st, stop=sp,
                        )
                        nc.tensor.matmul(
                            pv[:], wkv_sb[:, k * 256 + 128: k * 256 + 256], hsl,
                            start=st, stop=sp,
                        )
                        if k in (2, 4, 6, 8, 10, 12) and pending:
                            pending.pop(0)()
                    # evacuate this block; defer rope/transposes into next block
                    newpend = []
                    for m in range(HPC):
                        qsl = slice(m * S + sb_i * SB, m * S + (sb_i + 1) * SB)
                        nc.scalar.copy(qT[:, qsl], pq[m][:])
                        newpend.append(
                            lambda qsl=qsl, sl=sl: rope_emit(qT, qsl, sl)
                        )
                    nc.scalar.copy(kT[:, sl], pk[:])
                    newpend.append(lambda sl=sl: rope_emit(kT, sl, sl))
                    vtmp = vt.tile([128, SB], F32R, tag="vtmp")
                    nc.scalar.copy(vtmp[:], pv[:])

                    def emit_transposes(sb_i=sb_i, vtmp=vtmp):
                        for tt in range(4):
                            tp = psT.tile([128, 128], F32R, tag="tp")
                            nc.tensor.transpose(
                                tp[:], vtmp[:, tt * 128:(tt + 1) * 128], ident[:]
                            )
                            nc.vector.tensor_copy(
                                vnat[:, (sb_i * 4 + tt) * 128:
                                     (sb_i * 4 + tt + 1) * 128],
                                tp[:],
                            )
                    newpend.append(emit_transposes)
                    for fn in pending:   # anything not yet flushed
                        fn()
                    pending = newpend
                for fn in pending:
                    fn()
                pending = []

            # ---- Wo prefetch (lands well before stage 4 needs it) ----
            nc.sync.dma_start(out=wo_sb[:], in_=wo[:, :])

            # ---- stage 3+4: attention with interleaved output projection ----
            with (
                tc.tile_pool(name="at", bufs=2) as at,
                tc.tile_pool(name="ex", bufs=5) as ex,
                tc.tile_pool(name="rc", bufs=2) as rc,
                tc.tile_pool(name="ob", bufs=2) as ob,
                tc.tile_pool(name="psL", bufs=3, space="PSUM") as psL,
                tc.tile_pool(name="psO", bufs=2, space="PSUM") as psO,
                tc.tile_pool(name="psD", bufs=2, space="PSUM") as psD,
                tc.tile_pool(name="psW", bufs=1, space="PSUM") as psW,
            ):
                def stage4_chunk(qbx, tl, atile):
                    # out[q-chunk t, :] partial = attn(:, t-cols) @ Wo
                    t = qbx * 4 + tl
                    obuf = ob.tile([128, D], F32, tag="obuf")
                    for n4 in range(4):
                        pw = psW.tile([128, SB], F32, tag="pw")
                        for hh in range(HPC):
                            nc.tensor.matmul(
                                pw[:],
                                atile[:, hh * SB + tl * 128:
                                      hh * SB + (tl + 1) * 128],
                                wo_sb[:, hh * D + n4 * SB: hh * D + (n4 + 1) * SB],
                                start=(hh == 0), stop=(hh == HPC - 1),
                            )
                        osl = slice(n4 * SB, (n4 + 1) * SB)
                        if n4 % 2 == 0:
                            nc.scalar.copy(obuf[:, osl], pw[:])
                        else:
                            nc.vector.tensor_copy(obuf[:, osl], pw[:])
                    nc.sync.dma_start(
                        out=out[t * 128:(t + 1) * 128, :], in_=obuf[:]
                    )

                prev_attnT = None
                for qb in range(NSB):
                    q0 = qb * SB
                    kt_lo = max(0, q0 - WINDOW + 1) // 128
                    kt_hi = q0 // 128 + 3
                    attnT = at.tile([128, HPC * SB], F32R, tag="attnT",
                                    name=f"attnT_{qb}")
                    for h in range(HPC):
                        po = psO.tile([128, SB], F32, tag="po")
                        pd = psD.tile([128, SB], F32, tag="pd")
                        inflight = []

                        def flush_one():
                            kt, lo, hi, e = inflight.pop(0)
                            ksl = slice(kt * 128, (kt + 1) * 128)
                            st, sp = (kt == kt_lo), (kt == kt_hi)
                            nc.tensor.matmul(
                                po[:, lo:hi], vnat[:, ksl], e[:, lo:hi],
                                start=st, stop=sp,
                            )
                            nc.tensor.matmul(
                                pd[:, lo:hi], ones[:], e[:, lo:hi],
                                start=st, stop=sp,
                            )

                        for kt in range(kt_lo, kt_hi + 1):
                            dp = kt * 128 - q0
                            # valid cols [lo, hi): N even and >= 256 for the
                            # s3d3 fp32r full-rate ISA rules
                            lo = min(max(0, dp), SB - 256)
                            hi_i = dp + WINDOW + 127
                            hi = max(min(SB, hi_i + (hi_i & 1)), 256)
                            n = hi - lo
                            ksl = slice(kt * 128, (kt + 1) * 128)
                            pl = psL.tile([128, SB], F32, tag="pl")
                            nc.tensor.matmul(
                                pl[:, lo:hi], kT[:, ksl],
                                qT[:, h * S + q0 + lo: h * S + q0 + hi],
                                start=True, stop=True,
                            )
                            e = ex.tile([128, SB], F32R, tag="e")
                            nc.scalar.activation(
                                e[:, lo:hi], pl[:, lo:hi], ACTF.Exp, scale=SCALE
                            )
                            if dp >= 0:
                                # causal: keep j - i - dp >= 0 (local f = j - lo)
                                nc.gpsimd.affine_select(
                                    out=e[:, lo:hi], in_=e[:, lo:hi],
                                    pattern=[[1, n]], compare_op=ALU.is_ge,
                                    fill=0.0, base=lo - dp, channel_multiplier=-1,
                                )
                            elif dp <= -(WINDOW - SB + 1):
                                # window: keep i - j + dp + W-1 >= 0 (lo == 0)
                                nc.gpsimd.affine_select(
                                    out=e[:, lo:hi], in_=e[:, lo:hi],
                                    pattern=[[-1, n]], compare_op=ALU.is_ge,
                                    fill=0.0, base=WINDOW - 1 + dp,
                                    channel_multiplier=1,
                                )
                            inflight.append((kt, lo, hi, e))
                            if len(inflight) > PIPE:
                                flush_one()
                        while inflight:
                            flush_one()
                        rec = rc.tile([128, SB], F32, tag="rec")
                        nc.vector.reciprocal(rec[:], pd[:])
                        nc.vector.tensor_mul(
                            attnT[:, h * SB:(h + 1) * SB], po[:], rec[:]
                        )
                        if prev_attnT is not None:
                            stage4_chunk(qb - 1, h, prev_attnT)
                    prev_attnT = attnT
                for tl in range(4):
                    stage4_chunk(NSB - 1, tl, prev_attnT)
    _split_multiwaits(nc)
    return nc


_NC = None


def _get_nc():
    global _NC
    if _NC is None:
        _NC = build_nc()
    return _NC


def _make_in_maps(hidden, position_ids, Wqa, Wqb, Wk, Wv, Wo):
    hidden = np.asarray(hidden, dtype=np.float32)
    position_ids = np.asarray(position_ids)
    Wqa = np.asarray(Wqa, dtype=np.float32)
    Wqb = np.asarray(Wqb, dtype=np.float32)
    Wk = np.asarray(Wk, dtype=np.float32)
    Wv = np.asarray(Wv, dtype=np.float32)
    Wo = np.asarray(Wo, dtype=np.float32)
    weff_full = Wqa @ Wqb  # [D, H*HD]; exact assoc. fold of the LoRA Q proj

    inv_freq = 1.0 / (ROPE_BASE ** (np.arange(0, ROT, 2, dtype=np.float32) / ROT))
    in_maps = []
    for c in range(N_CORES):
        b, g = c // KVH, c % KVH
        pos = position_ids[b].astype(np.float32)
        freqs = pos[:, None] * inv_freq[None, :]        # [S, 32]
        cosT = np.cos(freqs).T.astype(np.float32)       # [32, S]
        sinT = np.sin(freqs).T.astype(np.float32)
        rcs = np.concatenate([cosT, cosT, sinT, sinT], axis=0)  # [128, S]
        hsb = (hidden[b].T.reshape(KT, 128, NSB, SB)
               .transpose(1, 2, 0, 3).reshape(128, NSB * KT * SB))
        weff = (weff_full[:, g * HPC * HD:(g + 1) * HPC * HD]
                .reshape(KT, 128, 512).transpose(1, 0, 2).reshape(128, KT * 512))
        wkv = np.concatenate(
            [Wk[:, g * HD:(g + 1) * HD], Wv[:, g * HD:(g + 1) * HD]], axis=1
        ).reshape(KT, 128, 256).transpose(1, 0, 2).reshape(128, KT * 256)
        wog = (Wo[g * HPC * HD:(g + 1) * HPC * HD, :]
               .reshape(HPC, 128, D).transpose(1, 0, 2).reshape(128, HPC * D))
        in_maps.append({
            "hid": np.ascontiguousarray(hsb),
            "weff": np.ascontiguousarray(weff),
            "wkv": np.ascontiguousarray(wkv),
            "wo": np.ascontiguousarray(wog),
            "rcs": np.ascontiguousarray(rcs),
        })
    return in_maps


def _run(inputs, trace=False):
    nc = _get_nc()
    in_maps = _make_in_maps(**inputs)
    res = run_bass_kernel_spmd(nc, in_maps, list(range(N_CORES)), trace=trace)
    out = np.zeros((B, S, D), dtype=np.float32)
    for c in range(N_CORES):
        out[c // KVH] += res.results[c]["out"]
    return out, res


def kernel(**inputs) -> np.ndarray:
    return _run(inputs, trace=False)[0]


# revision 12
# speedup vs baseline: 1.1722x; 1.0140x over previous
"""DeepseekV4-style attention (partial-RoPE LoRA-Q GQA sliding-window) on 8
Trainium2 NeuronCores.

Sharding: core c = 4*b + g handles batch b (of 2) and GQA group g (of 4):
q heads 4g..4g+3, kv head g, the matching column slices of Wq_eff/Wk/Wv and
row slice of Wo.  Each core computes a partial output; the host sums the four
partials per batch.

Design notes:
- LoRA Q projection folded on the host (W_eff = Wqa @ Wqb slice).
- Host packs hidden/weights into the exact SBUF layouts so each tensor needs
  a handful of large DMAs; startup is finely interleaved so the first matmul
  fires ~2us in.
- RoPE's half-swap is a signed 64x64 permutation matmul on the PE; rope and
  V-transpose emissions for block b are deferred into block b+1's k-loop so
  the PE FIFO is never blocked by a DVE-dependent instruction.
- Attention tiles narrowed at causal/window edges (N even, >=256 to satisfy
  s3d3 fp32r ISA rules at full rate); chunk loop is software-pipelined
  (QK of chunk c+2 is emitted before PV/pd of chunk c).
- Output projection for q-block qb-1 is emitted interleaved after each head
  of q-block qb, so stage-4 matmuls fill the PE while DVE finishes the
  softmax normalization of qb.
"""

import numpy as np
import ml_dtypes
import concourse.bass as bass
import concourse.mybir as mybir
import concourse.tile as tile
from concourse.bass_utils import run_bass_kernel_spmd

F32 = mybir.dt.float32
F32R = mybir.dt.float32r
BF16 = mybir.dt.bfloat16
ACTF = mybir.ActivationFunctionType
ALU = mybir.AluOpType

B, S, D = 2, 2048, 2048
H, KVH, HD = 16, 4, 128
ROT, LORA, WINDOW = 64, 512, 1024
ROPE_BASE = 10000.0
SCALE = HD ** -0.5

HPC = H // KVH          # 4 q heads per core
SB = 512                # free-dim block for matmuls
NSB = S // SB           # 4 seq blocks
KT = D // 128           # 16 contraction tiles over D
ST = S // 128           # 16 seq 128-chunks
N_CORES = 8
PIPE = 2                # attention chunk software-pipeline depth


def _split_multiwaits(nc):
    """This image's walrus accepts only one embedded SyncWait per instruction;
    split Tile's multi-wait sync_infos into standalone event-semaphore waits."""
    n = 0
    for func in nc.m.functions:
        for bb in func.blocks:
            insts = list(bb.instructions)
            out = []
            changed = False
            for inst in insts:
                si = inst.sync_info
                if si is not None and si.on_wait and len(si.on_wait) > 1:
                    waits = list(si.on_wait)
                    for w in waits[:-1]:
                        ev = mybir.InstEventSemaphore(
                            name=f"{inst.name}_wsplit_{n}", ins=[], outs=[]
                        )
                        ev.engine = inst.engine
                        ev.sync_info = mybir.SyncInfo(on_wait=[w], on_update=[])
                        out.append(ev)
                        n += 1
                    inst.sync_info = mybir.SyncInfo(
                        on_wait=[waits[-1]], on_update=list(si.on_update or [])
                    )
                    changed = True
                out.append(inst)
            if changed:
                bb.instructions = out
    return n


def build_nc():
    nc = bass.Bass()
    # host-packed layouts: hid col = blk*8192 + k*512 + c; weff col = k*512+c;
    # wkv col = k*256+c; wo col = h*2048+c
    hid = nc.dram_tensor("hid", [128, NSB * KT * SB], BF16, kind="ExternalInput")
    weff = nc.dram_tensor("weff", [128, KT * 512], BF16, kind="ExternalInput")
    wkv = nc.dram_tensor("wkv", [128, KT * 256], BF16, kind="ExternalInput")
    wo = nc.dram_tensor("wo", [128, HPC * D], BF16, kind="ExternalInput")
    rcs = nc.dram_tensor("rcs", [128, S], F32R, kind="ExternalInput")
    out = nc.dram_tensor("out", [S, D], F32, kind="ExternalOutput")

    with tile.TileContext(nc) as tc:
        with (
            tc.tile_pool(name="cst", bufs=1) as cst,
            tc.tile_pool(name="big", bufs=1) as big,
        ):
            # ---- small constants (engine-built, no DMA) ----
            onesf = cst.tile([128, 128], F32, tag="onesf")
            nc.vector.memset(onesf[:], 1.0)
            ones = cst.tile([128, 128], F32R, tag="ones")
            nc.vector.tensor_copy(ones[:], onesf[:])
            identf = cst.tile([128, 128], F32, tag="identf")
            nc.gpsimd.affine_select(
                out=identf[:], in_=onesf[:], pattern=[[1, 128]],
                compare_op=ALU.is_equal, fill=0.0, base=0, channel_multiplier=-1,
            )
            ident = cst.tile([128, 128], F32R, tag="ident")
            nc.vector.tensor_copy(ident[:], identf[:])
            # signed rope permutation P: P[32+i, i] = -1, P[j, 32+j] = +1
            negf = cst.tile([64, 32], F32, tag="negf")
            nc.vector.memset(negf[:], -1.0)
            posf = cst.tile([64, 32], F32, tag="posf")
            nc.vector.memset(posf[:], 1.0)
            permf = cst.tile([64, 64], F32, tag="permf")
            nc.gpsimd.affine_select(
                out=permf[:, 0:32], in_=negf[:], pattern=[[-1, 32]],
                compare_op=ALU.is_equal, fill=0.0, base=-32, channel_multiplier=1,
            )
            nc.gpsimd.affine_select(
                out=permf[:, 32:64], in_=posf[:], pattern=[[-1, 32]],
                compare_op=ALU.is_equal, fill=0.0, base=0, channel_multiplier=1,
            )
            perm = cst.tile([64, 64], F32R, tag="perm")
            nc.vector.tensor_copy(perm[:], permf[:])

            # ---- persistent activations ----
            qT = big.tile([128, HPC * S], F32R, tag="qT")    # per-head Q^T [hd, s]
            kT = big.tile([128, S], F32R, tag="kT")
            vnat = big.tile([128, S], F32R, tag="vnat")      # V rows, chunk t at cols t*128
            wo_sb = big.tile([128, HPC * D], BF16, tag="wo_sb")

            with (
                tc.tile_pool(name="wp", bufs=1) as wp,
                tc.tile_pool(name="hp", bufs=5) as hp,
                tc.tile_pool(name="vt", bufs=2) as vt,
                tc.tile_pool(name="rp", bufs=2) as rp,
                tc.tile_pool(name="psA", bufs=1, space="PSUM") as psA,
                tc.tile_pool(name="psT", bufs=1, space="PSUM") as psT,
                tc.tile_pool(name="psP", bufs=1, space="PSUM") as psP,
            ):
                weff_sb = wp.tile([128, KT * 512], BF16, tag="weff_sb")
                wkv_sb = wp.tile([128, KT * 256], BF16, tag="wkv_sb")
                ropeCC = wp.tile([64, S], F32R, tag="ropeCC")
                ropeSS = wp.tile([64, S], F32R, tag="ropeSS")

                def dma_w(g0, g1):
                    nc.sync.dma_start(
                        out=weff_sb[:, g0 * 512:g1 * 512],
                        in_=weff[:, g0 * 512:g1 * 512],
                    )
                    nc.sync.dma_start(
                        out=wkv_sb[:, g0 * 256:g1 * 256],
                        in_=wkv[:, g0 * 256:g1 * 256],
                    )

                def dma_ht(sb_i, qd):
                    t = hp.tile([128, 2048], BF16, tag="ht", name=f"ht_{sb_i}_{qd}")
                    nc.sync.dma_start(
                        out=t[:],
                        in_=hid[:, sb_i * 8192 + qd * 2048:
                                sb_i * 8192 + (qd + 1) * 2048],
                    )
                    return t

                # startup interleave: weight k-groups racing block-0 hidden
                h0t = []
                dma_w(0, 1)
                t00 = hp.tile([128, 2048], BF16, tag="ht", name="ht_0_0")
                nc.sync.dma_start(out=t00[:, 0:512], in_=hid[:, 0:512])
                h0t.append(t00)
                dma_w(1, 4)
                nc.sync.dma_start(out=t00[:, 512:2048], in_=hid[:, 512:2048])
                h0t.append(dma_ht(0, 1))
                dma_w(4, 8)
                h0t.append(dma_ht(0, 2))
                dma_w(8, 12)
                h0t.append(dma_ht(0, 3))
                dma_w(12, 16)
                nc.sync.dma_start(out=ropeCC[:], in_=rcs[0:64, :])
                nc.sync.dma_start(out=ropeSS[:], in_=rcs[64:128, :])

                def rope_emit(dst, csl, rsl):
                    # dst rows 0:64 hold [x1; x2]; out = x*cos + P^T(x*sin)
                    tsin = rp.tile([64, SB], F32R, tag="tsin")
                    nc.vector.tensor_mul(tsin[:], dst[0:64, csl], ropeSS[:, rsl])
                    csb = rp.tile([64, SB], F32R, tag="csb")
                    nc.vector.tensor_mul(csb[:], dst[0:64, csl], ropeCC[:, rsl])
                    pP = psP.tile([64, SB], F32, tag="pP")
                    nc.tensor.matmul(pP[:], perm[:], tsin[:], start=True, stop=True)
                    nc.vector.tensor_add(dst[0:64, csl], csb[:], pP[:])

                # ---- stage 1: q^T (folded), k^T, v -> vnat ----
                pending = []   # deferred rope/transpose emissions from prev block
                for sb_i in range(NSB):
                    sl = slice(sb_i * SB, (sb_i + 1) * SB)
                    if sb_i > 0:
                        hts = [dma_ht(sb_i, qd) for qd in range(4)]
                    else:
                        hts = h0t
                    pq = [
                        psA.tile([128, SB], F32, tag=f"pq{m}", name=f"pq{m}_{sb_i}")
                        for m in range(HPC)
                    ]
                    pk = psA.tile([128, SB], F32, tag="pk")
                    pv = psA.tile([128, SB], F32, tag="pv")
                    for k in range(KT):
                        hsl = hts[k // 4][:, (k % 4) * 512:(k % 4 + 1) * 512]
                        st, sp = (k == 0), (k == KT - 1)
                        for m in range(HPC):
                            nc.tensor.matmul(
                                pq[m][:],
                                weff_sb[:, k * 512 + m * 128: k * 512 + (m + 1) * 128],
                                hsl, start=st, stop=sp,
                            )
                        nc.tensor.matmul(
                            pk[:], wkv_sb[:, k * 256: k * 256 + 128], hsl,
                            start=st, stop=sp,
                        )
                        nc.tensor.matmul(
                            pv[:], wkv_sb[:, k * 256 + 128: k * 256 + 256], hsl,
                            start=st, stop=sp,
                        )
                        if k in (2, 4, 6, 8, 10, 12) and pending:
                            pending.pop(0)()
                    # evacuate this block; defer rope/transposes into next block
                    newpend = []
                    for m in range(HPC):
                        qsl = slice(m * S + sb_i * SB, m * S + (sb_i + 1) * SB)
                        nc.scalar.copy(qT[:, qsl], pq[m][:])
                        newpend.append(
                            lambda qsl=qsl, sl=sl: rope_emit(qT, qsl, sl)
                        )
                    nc.scalar.copy(kT[:, sl], pk[:])
                    newpend.append(lambda sl=sl: rope_emit(kT, sl, sl))
                    vtmp = vt.tile([128, SB], F32R, tag="vtmp")
                    nc.scalar.copy(vtmp[:], pv[:])

                    def emit_transposes(sb_i=sb_i, vtmp=vtmp):
                        for tt in range(4):
                            tp = psT.tile([128, 128], F32R, tag="tp")
                            nc.tensor.transpose(
                                tp[:], vtmp[:, tt * 128:(tt + 1) * 128], ident[:]
                            )
                            nc.vector.tensor_copy(
                                vnat[:, (sb_i * 4 + tt) * 128:
                                     (sb_i * 4 + tt + 1) * 128],
                                tp[:],
                            )
                    newpend.append(emit_transposes)
                    for fn in pending:   # anything not yet flushed
                        fn()
                    pending = newpend
                for fn in pending:
                    fn()
                pending = []

            # ---- Wo prefetch (lands well before stage 4 needs it) ----
            nc.sync.dma_start(out=wo_sb[:], in_=wo[:, :])

            # ---- stage 3+4: attention with interleaved output projection ----
            with (
                tc.tile_pool(name="at", bufs=2) as at,
                tc.tile_pool(name="ex", bufs=5) as ex,
                tc.tile_pool(name="rc", bufs=2) as rc,
                tc.tile_pool(name="ob", bufs=2) as ob,
                tc.tile_pool(name="psL", bufs=3, space="PSUM") as psL,
                tc.tile_pool(name="psO", bufs=2, space="PSUM") as psO,
                tc.tile_pool(name="psD", bufs=2, space="PSUM") as psD,
                tc.tile_pool(name="psW", bufs=1, space="PSUM") as psW,
            ):
                def stage4_chunk(qbx, tl, atile):
                    # out[q-chunk t, :] partial = attn(:, t-cols) @ Wo
                    t = qbx * 4 + tl
                    obuf = ob.tile([128, D], F32, tag="obuf")
                    for n4 in range(4):
                        pw = psW.tile([128, SB], F32, tag="pw")
                        for hh in range(HPC):
                            nc.tensor.matmul(
                                pw[:],
                                atile[:, hh * SB + tl * 128:
                                      hh * SB + (tl + 1) * 128],
                                wo_sb[:, hh * D + n4 * SB: hh * D + (n4 + 1) * SB],
                                start=(hh == 0), stop=(hh == HPC - 1),
                            )
                        o0 = n4 * SB
                        nc.scalar.copy(obuf[:, o0:o0 + 256], pw[:, 0:256])
                        nc.vector.tensor_copy(
                            obuf[:, o0 + 256:o0 + SB], pw[:, 256:SB]
                        )
                        if n4 == 1:
                            nc.sync.dma_start(
                                out=out[t * 128:(t + 1) * 128, 0:2 * SB],
                                in_=obuf[:, 0:2 * SB],
                            )
                        elif n4 == 3:
                            nc.sync.dma_start(
                                out=out[t * 128:(t + 1) * 128, 2 * SB:D],
                                in_=obuf[:, 2 * SB:D],
                            )

                prev_attnT = None
                for qb in range(NSB):
                    q0 = qb * SB
                    kt_lo = max(0, q0 - WINDOW + 1) // 128
                    kt_hi = q0 // 128 + 3
                    attnT = at.tile([128, HPC * SB], BF16, tag="attnT",
                                    name=f"attnT_{qb}")
                    for h in range(HPC):
                        po = psO.tile([128, SB], F32, tag="po")
                        pd = psD.tile([128, SB], F32, tag="pd")
                        inflight = []

                        def flush_one():
                            kt, lo, hi, e = inflight.pop(0)
                            ksl = slice(kt * 128, (kt + 1) * 128)
                            st, sp = (kt == kt_lo), (kt == kt_hi)
                            nc.tensor.matmul(
                                po[:, lo:hi], vnat[:, ksl], e[:, lo:hi],
                                start=st, stop=sp,
                            )
                            nc.tensor.matmul(
                                pd[:, lo:hi], ones[:], e[:, lo:hi],
                                start=st, stop=sp,
                            )

                        for kt in range(kt_lo, kt_hi + 1):
                            dp = kt * 128 - q0
                            # valid cols [lo, hi): N even and >= 256 for the
                            # s3d3 fp32r full-rate ISA rules
                            lo = min(max(0, dp), SB - 256)
                            hi_i = dp + WINDOW + 127
                            hi = max(min(SB, hi_i + (hi_i & 1)), 256)
                            n = hi - lo
                            ksl = slice(kt * 128, (kt + 1) * 128)
                            pl = psL.tile([128, SB], F32, tag="pl")
                            nc.tensor.matmul(
                                pl[:, lo:hi], kT[:, ksl],
                                qT[:, h * S + q0 + lo: h * S + q0 + hi],
                                start=True, stop=True,
                            )
                            e = ex.tile([128, SB], F32R, tag="e")
                            nc.scalar.activation(
                                e[:, lo:hi], pl[:, lo:hi], ACTF.Exp, scale=SCALE
                            )
                            if dp >= 0:
                                # causal: keep j - i - dp >= 0 (local f = j - lo)
                                nc.gpsimd.affine_select(
                                    out=e[:, lo:hi], in_=e[:, lo:hi],
                                    pattern=[[1, n]], compare_op=ALU.is_ge,
                                    fill=0.0, base=lo - dp, channel_multiplier=-1,
                                )
                            elif dp <= -(WINDOW - SB + 1):
                                # window: keep i - j + dp + W-1 >= 0 (lo == 0)
                                nc.gpsimd.affine_select(
                                    out=e[:, lo:hi], in_=e[:, lo:hi],
                                    pattern=[[-1, n]], compare_op=ALU.is_ge,
                                    fill=0.0, base=WINDOW - 1 + dp,
                                    channel_multiplier=1,
                                )
                            inflight.append((kt, lo, hi, e))
                            if len(inflight) > PIPE:
                                flush_one()
                        while inflight:
                            flush_one()
                        rec = rc.tile([128, SB], F32, tag="rec")
                        nc.vector.reciprocal(rec[:], pd[:])
                        nc.vector.tensor_mul(
                            attnT[:, h * SB:(h + 1) * SB], po[:], rec[:]
                        )
                        if prev_attnT is not None:
                            stage4_chunk(qb - 1, h, prev_attnT)
                    prev_attnT = attnT
                for tl in range(4):
                    stage4_chunk(NSB - 1, tl, prev_attnT)
    _split_multiwaits(nc)
    return nc


_NC = None


def _get_nc():
    global _NC
    if _NC is None:
        _NC = build_nc()
    return _NC


def _make_in_maps(hidden, position_ids, Wqa, Wqb, Wk, Wv, Wo):
    hidden = np.asarray(hidden, dtype=np.float32)
    position_ids = np.asarray(position_ids)
    Wqa = np.asarray(Wqa, dtype=np.float32)
    Wqb = np.asarray(Wqb, dtype=np.float32)
    Wk = np.asarray(Wk, dtype=np.float32)
    Wv = np.asarray(Wv, dtype=np.float32)
    Wo = np.asarray(Wo, dtype=np.float32)
    weff_full = Wqa @ Wqb  # [D, H*HD]; exact assoc. fold of the LoRA Q proj

    inv_freq = 1.0 / (ROPE_BASE ** (np.arange(0, ROT, 2, dtype=np.float32) / ROT))
    in_maps = []
    for c in range(N_CORES):
        b, g = c // KVH, c % KVH
        pos = position_ids[b].astype(np.float32)
        freqs = pos[:, None] * inv_freq[None, :]        # [S, 32]
        cosT = np.cos(freqs).T.astype(np.float32)       # [32, S]
        sinT = np.sin(freqs).T.astype(np.float32)
        rcs = np.concatenate([cosT, cosT, sinT, sinT], axis=0)  # [128, S]
        hsb = (hidden[b].T.reshape(KT, 128, NSB, SB)
               .transpose(1, 2, 0, 3).reshape(128, NSB * KT * SB))
        weff = (weff_full[:, g * HPC * HD:(g + 1) * HPC * HD]
                .reshape(KT, 128, 512).transpose(1, 0, 2).reshape(128, KT * 512))
        wkv = np.concatenate(
            [Wk[:, g * HD:(g + 1) * HD], Wv[:, g * HD:(g + 1) * HD]], axis=1
        ).reshape(KT, 128, 256).transpose(1, 0, 2).reshape(128, KT * 256)
        wog = (Wo[g * HPC * HD:(g + 1) * HPC * HD, :]
               .reshape(HPC, 128, D).transpose(1, 0, 2).reshape(128, HPC * D))
        in_maps.append({
            "hid": np.ascontiguousarray(hsb.astype(ml_dtypes.bfloat16)),
            "weff": np.ascontiguousarray(weff.astype(ml_dtypes.bfloat16)),
            "wkv": np.ascontiguousarray(wkv.astype(ml_dtypes.bfloat16)),
            "wo": np.ascontiguousarray(wog.astype(ml_dtypes.bfloat16)),
            "rcs": np.ascontiguousarray(rcs),
        })
    return in_maps


def _run(inputs, trace=False):
    nc = _get_nc()
    in_maps = _make_in_maps(**inputs)
    res = run_bass_kernel_spmd(nc, in_maps, list(range(N_CORES)), trace=trace)
    out = np.zeros((B, S, D), dtype=np.float32)
    for c in range(N_CORES):
        out[c // KVH] += res.results[c]["out"]
    return out, res


def kernel(**inputs) -> np.ndarray:
    return _run(inputs, trace=False)[0]


# revision 13
# speedup vs baseline: 1.4006x; 1.1948x over previous
"""DeepseekV4-style attention (partial-RoPE LoRA-Q GQA sliding-window) on 8
Trainium2 NeuronCores.

Sharding: core c = 4*b + g handles batch b (of 2) and GQA group g (of 4):
q heads 4g..4g+3, kv head g, the matching column slices of Wq_eff/Wk/Wv and
row slice of Wo.  Each core computes a partial output; the host sums the four
partials per batch.

Design notes:
- LoRA Q projection folded on the host (W_eff = Wqa @ Wqb slice).
- Host packs hidden/weights into the exact SBUF layouts so each tensor needs
  a handful of large DMAs; startup is finely interleaved so the first matmul
  fires ~2us in.
- RoPE's half-swap is a signed 64x64 permutation matmul on the PE; rope and
  V-transpose emissions for block b are deferred into block b+1's k-loop so
  the PE FIFO is never blocked by a DVE-dependent instruction.
- Attention tiles narrowed at causal/window edges (N even, >=256 to satisfy
  s3d3 fp32r ISA rules at full rate); chunk loop is software-pipelined
  (QK of chunk c+2 is emitted before PV/pd of chunk c).
- Output projection for q-block qb-1 is emitted interleaved after each head
  of q-block qb, so stage-4 matmuls fill the PE while DVE finishes the
  softmax normalization of qb.
"""

import numpy as np
import ml_dtypes
import concourse.bass as bass
import concourse.mybir as mybir
import concourse.tile as tile
from concourse.bass_utils import run_bass_kernel_spmd

F32 = mybir.dt.float32
F32R = mybir.dt.float32r
BF16 = mybir.dt.bfloat16
ACTF = mybir.ActivationFunctionType
ALU = mybir.AluOpType

B, S, D = 2, 2048, 2048
H, KVH, HD = 16, 4, 128
ROT, LORA, WINDOW = 64, 512, 1024
ROPE_BASE = 10000.0
SCALE = HD ** -0.5

HPC = H // KVH          # 4 q heads per core
SB = 512                # free-dim block for matmuls
NSB = S // SB           # 4 seq blocks
KT = D // 128           # 16 contraction tiles over D
ST = S // 128           # 16 seq 128-chunks
N_CORES = 8
PIPE = 2                # attention chunk software-pipeline depth


def _split_multiwaits(nc):
    """This image's walrus accepts only one embedded SyncWait per instruction;
    split Tile's multi-wait sync_infos into standalone event-semaphore waits."""
    n = 0
    for func in nc.m.functions:
        for bb in func.blocks:
            insts = list(bb.instructions)
            out = []
            changed = False
            for inst in insts:
                si = inst.sync_info
                if si is not None and si.on_wait and len(si.on_wait) > 1:
                    waits = list(si.on_wait)
                    for w in waits[:-1]:
                        ev = mybir.InstEventSemaphore(
                            name=f"{inst.name}_wsplit_{n}", ins=[], outs=[]
                        )
                        ev.engine = inst.engine
                        ev.sync_info = mybir.SyncInfo(on_wait=[w], on_update=[])
                        out.append(ev)
                        n += 1
                    inst.sync_info = mybir.SyncInfo(
                        on_wait=[waits[-1]], on_update=list(si.on_update or [])
                    )
                    changed = True
                out.append(inst)
            if changed:
                bb.instructions = out
    return n


def build_nc():
    nc = bass.Bass()
    # host-packed layouts: hid col = blk*8192 + k*512 + c; weff col = k*512+c;
    # wkv col = k*256+c; wo col = h*2048+c
    hid = nc.dram_tensor("hid", [128, NSB * KT * SB], BF16, kind="ExternalInput")
    weff = nc.dram_tensor("weff", [128, KT * 512], BF16, kind="ExternalInput")
    wkv = nc.dram_tensor("wkv", [128, KT * 256], BF16, kind="ExternalInput")
    wo = nc.dram_tensor("wo", [128, HPC * D], BF16, kind="ExternalInput")
    rcs = nc.dram_tensor("rcs", [128, S], F32R, kind="ExternalInput")
    out = nc.dram_tensor("out", [S, D], F32, kind="ExternalOutput")

    with tile.TileContext(nc) as tc:
        with (
            tc.tile_pool(name="cst", bufs=1) as cst,
            tc.tile_pool(name="big", bufs=1) as big,
        ):
            # ---- small constants (engine-built, no DMA) ----
            onesf = cst.tile([128, 128], F32, tag="onesf")
            nc.vector.memset(onesf[:], 1.0)
            ones = cst.tile([128, 128], F32R, tag="ones")
            nc.vector.tensor_copy(ones[:], onesf[:])
            identf = cst.tile([128, 128], F32, tag="identf")
            nc.gpsimd.affine_select(
                out=identf[:], in_=onesf[:], pattern=[[1, 128]],
                compare_op=ALU.is_equal, fill=0.0, base=0, channel_multiplier=-1,
            )
            ident = cst.tile([128, 128], F32R, tag="ident")
            nc.vector.tensor_copy(ident[:], identf[:])
            # signed rope permutation P: P[32+i, i] = -1, P[j, 32+j] = +1
            negf = cst.tile([64, 32], F32, tag="negf")
            nc.vector.memset(negf[:], -1.0)
            posf = cst.tile([64, 32], F32, tag="posf")
            nc.vector.memset(posf[:], 1.0)
            permf = cst.tile([64, 64], F32, tag="permf")
            nc.gpsimd.affine_select(
                out=permf[:, 0:32], in_=negf[:], pattern=[[-1, 32]],
                compare_op=ALU.is_equal, fill=0.0, base=-32, channel_multiplier=1,
            )
            nc.gpsimd.affine_select(
                out=permf[:, 32:64], in_=posf[:], pattern=[[-1, 32]],
                compare_op=ALU.is_equal, fill=0.0, base=0, channel_multiplier=1,
            )
            perm = cst.tile([64, 64], F32R, tag="perm")
            nc.vector.tensor_copy(perm[:], permf[:])

            # ---- persistent activations ----
            qT = big.tile([128, HPC * S], F32R, tag="qT")    # per-head Q^T [hd, s]
            kT = big.tile([128, S], F32R, tag="kT")
            vnat = big.tile([128, S], F32R, tag="vnat")      # V rows, chunk t at cols t*128
            wo_sb = big.tile([128, HPC * D], BF16, tag="wo_sb")

            with (
                tc.tile_pool(name="wp", bufs=1) as wp,
                tc.tile_pool(name="hp", bufs=5) as hp,
                tc.tile_pool(name="vt", bufs=2) as vt,
                tc.tile_pool(name="rp", bufs=2) as rp,
                tc.tile_pool(name="psA", bufs=1, space="PSUM") as psA,
                tc.tile_pool(name="psT", bufs=1, space="PSUM") as psT,
                tc.tile_pool(name="psP", bufs=1, space="PSUM") as psP,
            ):
                weff_sb = wp.tile([128, KT * 512], BF16, tag="weff_sb")
                wkv_sb = wp.tile([128, KT * 256], BF16, tag="wkv_sb")
                ropeCC = wp.tile([64, S], F32R, tag="ropeCC")
                ropeSS = wp.tile([64, S], F32R, tag="ropeSS")

                def dma_w(g0, g1):
                    nc.sync.dma_start(
                        out=weff_sb[:, g0 * 512:g1 * 512],
                        in_=weff[:, g0 * 512:g1 * 512],
                    )
                    nc.sync.dma_start(
                        out=wkv_sb[:, g0 * 256:g1 * 256],
                        in_=wkv[:, g0 * 256:g1 * 256],
                    )

                def dma_ht(sb_i, qd):
                    t = hp.tile([128, 2048], BF16, tag="ht", name=f"ht_{sb_i}_{qd}")
                    nc.sync.dma_start(
                        out=t[:],
                        in_=hid[:, sb_i * 8192 + qd * 2048:
                                sb_i * 8192 + (qd + 1) * 2048],
                    )
                    return t

                # startup interleave: weight k-groups racing block-0 hidden
                h0t = []
                dma_w(0, 1)
                t00 = hp.tile([128, 2048], BF16, tag="ht", name="ht_0_0")
                nc.sync.dma_start(out=t00[:, 0:512], in_=hid[:, 0:512])
                h0t.append(t00)
                dma_w(1, 4)
                nc.sync.dma_start(out=t00[:, 512:2048], in_=hid[:, 512:2048])
                h0t.append(dma_ht(0, 1))
                dma_w(4, 8)
                h0t.append(dma_ht(0, 2))
                dma_w(8, 12)
                h0t.append(dma_ht(0, 3))
                dma_w(12, 16)
                nc.sync.dma_start(out=ropeCC[:], in_=rcs[0:64, :])
                nc.sync.dma_start(out=ropeSS[:], in_=rcs[64:128, :])

                def rope_emit(dst, csl, rsl):
                    # dst rows 0:64 hold [x1; x2]; out = x*cos + P^T(x*sin)
                    tsin = rp.tile([64, SB], F32R, tag="tsin")
                    nc.vector.tensor_mul(tsin[:], dst[0:64, csl], ropeSS[:, rsl])
                    csb = rp.tile([64, SB], F32R, tag="csb")
                    nc.vector.tensor_mul(csb[:], dst[0:64, csl], ropeCC[:, rsl])
                    pP = psP.tile([64, SB], F32, tag="pP")
                    nc.tensor.matmul(pP[:], perm[:], tsin[:], start=True, stop=True)
                    nc.vector.tensor_add(dst[0:64, csl], csb[:], pP[:])

                # ---- stage 1: q^T (folded), k^T, v -> vnat ----
                pending = []   # deferred rope/transpose emissions from prev block
                for sb_i in range(NSB):
                    sl = slice(sb_i * SB, (sb_i + 1) * SB)
                    if sb_i > 0:
                        hts = [dma_ht(sb_i, qd) for qd in range(4)]
                    else:
                        hts = h0t
                    pq = [
                        psA.tile([128, SB], F32, tag=f"pq{m}", name=f"pq{m}_{sb_i}")
                        for m in range(HPC)
                    ]
                    pk = psA.tile([128, SB], F32, tag="pk")
                    pv = psA.tile([128, SB], F32, tag="pv")
                    for k in range(KT):
                        hsl = hts[k // 4][:, (k % 4) * 512:(k % 4 + 1) * 512]
                        st, sp = (k == 0), (k == KT - 1)
                        for m in range(HPC):
                            nc.tensor.matmul(
                                pq[m][:],
                                weff_sb[:, k * 512 + m * 128: k * 512 + (m + 1) * 128],
                                hsl, start=st, stop=sp,
                            )
                        nc.tensor.matmul(
                            pk[:], wkv_sb[:, k * 256: k * 256 + 128], hsl,
                            start=st, stop=sp,
                        )
                        nc.tensor.matmul(
                            pv[:], wkv_sb[:, k * 256 + 128: k * 256 + 256], hsl,
                            start=st, stop=sp,
                        )
                        if k in (2, 4, 6, 8, 10, 12) and pending:
                            pending.pop(0)()
                    # evacuate this block; defer rope/transposes into next block
                    newpend = []
                    for m in range(HPC):
                        qsl = slice(m * S + sb_i * SB, m * S + (sb_i + 1) * SB)
                        nc.scalar.copy(qT[:, qsl], pq[m][:])
                        newpend.append(
                            lambda qsl=qsl, sl=sl: rope_emit(qT, qsl, sl)
                        )
                    nc.scalar.copy(kT[:, sl], pk[:])
                    newpend.append(lambda sl=sl: rope_emit(kT, sl, sl))
                    vtmp = vt.tile([128, SB], F32R, tag="vtmp")
                    nc.scalar.copy(vtmp[:], pv[:])

                    def emit_transposes(sb_i=sb_i, vtmp=vtmp):
                        for tt in range(4):
                            tp = psT.tile([128, 128], F32R, tag="tp")
                            nc.tensor.transpose(
                                tp[:], vtmp[:, tt * 128:(tt + 1) * 128], ident[:]
                            )
                            nc.vector.tensor_copy(
                                vnat[:, (sb_i * 4 + tt) * 128:
                                     (sb_i * 4 + tt + 1) * 128],
                                tp[:],
                            )
                    newpend.append(emit_transposes)
                    for fn in pending:   # anything not yet flushed
                        fn()
                    pending = newpend
                for fn in pending:
                    fn()
                pending = []

            # ---- Wo prefetch (lands well before stage 4 needs it) ----
            nc.sync.dma_start(out=wo_sb[:], in_=wo[:, :])

            # ---- stage 3+4: attention with interleaved output projection ----
            with (
                tc.tile_pool(name="at", bufs=2) as at,
                tc.tile_pool(name="ex", bufs=5) as ex,
                tc.tile_pool(name="rc", bufs=2) as rc,
                tc.tile_pool(name="ob", bufs=2) as ob,
                tc.tile_pool(name="psL", bufs=3, space="PSUM") as psL,
                tc.tile_pool(name="psO", bufs=2, space="PSUM") as psO,
                tc.tile_pool(name="psD", bufs=2, space="PSUM") as psD,
                tc.tile_pool(name="psW", bufs=1, space="PSUM") as psW,
            ):
                def stage4_chunk(qbx, tl, atile):
                    # out[q-chunk t, :] partial = attn(:, t-cols) @ Wo
                    t = qbx * 4 + tl
                    obuf = ob.tile([128, D], F32, tag="obuf")
                    for n4 in range(4):
                        pw = psW.tile([128, SB], F32, tag="pw")
                        for hh in range(HPC):
                            nc.tensor.matmul(
                                pw[:],
                                atile[:, hh * SB + tl * 128:
                                      hh * SB + (tl + 1) * 128],
                                wo_sb[:, hh * D + n4 * SB: hh * D + (n4 + 1) * SB],
                                start=(hh == 0), stop=(hh == HPC - 1),
                            )
                        o0 = n4 * SB
                        nc.scalar.copy(obuf[:, o0:o0 + SB], pw[:])
                        if n4 == 1:
                            nc.sync.dma_start(
                                out=out[t * 128:(t + 1) * 128, 0:2 * SB],
                                in_=obuf[:, 0:2 * SB],
                            )
                        elif n4 == 3:
                            nc.sync.dma_start(
                                out=out[t * 128:(t + 1) * 128, 2 * SB:D],
                                in_=obuf[:, 2 * SB:D],
                            )

                prev_attnT = None
                for qb in range(NSB):
                    q0 = qb * SB
                    kt_lo = max(0, q0 - WINDOW + 1) // 128
                    kt_hi = q0 // 128 + 3
                    attnT = at.tile([128, HPC * SB], BF16, tag="attnT",
                                    name=f"attnT_{qb}")
                    for h in range(HPC):
                        po = psO.tile([128, SB], F32, tag="po")
                        pd = psD.tile([128, SB], F32, tag="pd")
                        inflight = []

                        def flush_one():
                            kt, lo, hi, e = inflight.pop(0)
                            ksl = slice(kt * 128, (kt + 1) * 128)
                            st, sp = (kt == kt_lo), (kt == kt_hi)
                            nc.tensor.matmul(
                                po[:, lo:hi], vnat[:, ksl], e[:, lo:hi],
                                start=st, stop=sp,
                            )
                            nc.tensor.matmul(
                                pd[:, lo:hi], ones[:], e[:, lo:hi],
                                start=st, stop=sp,
                            )

                        for kt in range(kt_lo, kt_hi + 1):
                            dp = kt * 128 - q0
                            # valid cols [lo, hi): N even and >= 256 for the
                            # s3d3 fp32r full-rate ISA rules
                            lo = min(max(0, dp), SB - 256)
                            hi_i = dp + WINDOW + 127
                            hi = max(min(SB, hi_i + (hi_i & 1)), 256)
                            n = hi - lo
                            ksl = slice(kt * 128, (kt + 1) * 128)
                            pl = psL.tile([128, SB], F32, tag="pl")
                            nc.tensor.matmul(
                                pl[:, lo:hi], kT[:, ksl],
                                qT[:, h * S + q0 + lo: h * S + q0 + hi],
                                start=True, stop=True,
                            )
                            e = ex.tile([128, SB], F32R, tag="e")
                            nc.scalar.activation(
                                e[:, lo:hi], pl[:, lo:hi], ACTF.Exp, scale=SCALE
                            )
                            if dp >= 0:
                                # causal: keep j - i - dp >= 0 (local f = j - lo)
                                nc.gpsimd.affine_select(
                                    out=e[:, lo:hi], in_=e[:, lo:hi],
                                    pattern=[[1, n]], compare_op=ALU.is_ge,
                                    fill=0.0, base=lo - dp, channel_multiplier=-1,
                                )
                            elif dp <= -(WINDOW - SB + 1):
                                # window: keep i - j + dp + W-1 >= 0 (lo == 0)
                                nc.gpsimd.affine_select(
                                    out=e[:, lo:hi], in_=e[:, lo:hi],
                                    pattern=[[-1, n]], compare_op=ALU.is_ge,
                                    fill=0.0, base=WINDOW - 1 + dp,
                                    channel_multiplier=1,
                                )
                            inflight.append((kt, lo, hi, e))
                            if len(inflight) > PIPE:
                                flush_one()
                        while inflight:
                            flush_one()
                        rec = rc.tile([128, SB], F32, tag="rec")
                        nc.vector.reciprocal(rec[:], pd[:])
                        nc.vector.tensor_mul(
                            attnT[:, h * SB:(h + 1) * SB], po[:], rec[:]
                        )
                        if prev_attnT is not None:
                            stage4_chunk(qb - 1, h, prev_attnT)
                    prev_attnT = attnT
                for tl in range(4):
                    stage4_chunk(NSB - 1, tl, prev_attnT)
    _split_multiwaits(nc)
    return nc


_NC = None


def _get_nc():
    global _NC
    if _NC is None:
        _NC = build_nc()
    return _NC


def _make_in_maps(hidden, position_ids, Wqa, Wqb, Wk, Wv, Wo):
    hidden = np.asarray(hidden, dtype=np.float32)
    position_ids = np.asarray(position_ids)
    Wqa = np.asarray(Wqa, dtype=np.float32)
    Wqb = np.asarray(Wqb, dtype=np.float32)
    Wk = np.asarray(Wk, dtype=np.float32)
    Wv = np.asarray(Wv, dtype=np.float32)
    Wo = np.asarray(Wo, dtype=np.float32)
    weff_full = Wqa @ Wqb  # [D, H*HD]; exact assoc. fold of the LoRA Q proj

    inv_freq = 1.0 / (ROPE_BASE ** (np.arange(0, ROT, 2, dtype=np.float32) / ROT))
    in_maps = []
    for c in range(N_CORES):
        b, g = c // KVH, c % KVH
        pos = position_ids[b].astype(np.float32)
        freqs = pos[:, None] * inv_freq[None, :]        # [S, 32]
        cosT = np.cos(freqs).T.astype(np.float32)       # [32, S]
        sinT = np.sin(freqs).T.astype(np.float32)
        rcs = np.concatenate([cosT, cosT, sinT, sinT], axis=0)  # [128, S]
        hsb = (hidden[b].T.reshape(KT, 128, NSB, SB)
               .transpose(1, 2, 0, 3).reshape(128, NSB * KT * SB))
        weff = (weff_full[:, g * HPC * HD:(g + 1) * HPC * HD]
                .reshape(KT, 128, 512).transpose(1, 0, 2).reshape(128, KT * 512))
        wkv = np.concatenate(
            [Wk[:, g * HD:(g + 1) * HD], Wv[:, g * HD:(g + 1) * HD]], axis=1
        ).reshape(KT, 128, 256).transpose(1, 0, 2).reshape(128, KT * 256)
        wog = (Wo[g * HPC * HD:(g + 1) * HPC * HD, :]
               .reshape(HPC, 128, D).transpose(1, 0, 2).reshape(128, HPC * D))
        in_maps.append({
            "hid": np.ascontiguousarray(hsb.astype(ml_dtypes.bfloat16)),
            "weff": np.ascontiguousarray(weff.astype(ml_dtypes.bfloat16)),
            "wkv": np.ascontiguousarray(wkv.astype(ml_dtypes.bfloat16)),
            "wo": np.ascontiguousarray(wog.astype(ml_dtypes.bfloat16)),
            "rcs": np.ascontiguousarray(rcs),
        })
    return in_maps


def _run(inputs, trace=False):
    nc = _get_nc()
    in_maps = _make_in_maps(**inputs)
    res = run_bass_kernel_spmd(nc, in_maps, list(range(N_CORES)), trace=trace)
    out = np.zeros((B, S, D), dtype=np.float32)
    for c in range(N_CORES):
        out[c // KVH] += res.results[c]["out"]
    return out, res


def kernel(**inputs) -> np.ndarray:
    return _run(inputs, trace=False)[0]


# revision 14
# speedup vs baseline: 1.4082x; 1.0054x over previous
"""DeepseekV4-style attention (partial-RoPE LoRA-Q GQA sliding-window) on 8
Trainium2 NeuronCores.

Sharding: core c = 4*b + g handles batch b (of 2) and GQA group g (of 4):
q heads 4g..4g+3, kv head g, the matching column slices of Wq_eff/Wk/Wv and
row slice of Wo.  Each core computes a partial output; the host sums the four
partials per batch.

Design notes:
- LoRA Q projection folded on the host (W_eff = Wqa @ Wqb slice).
- Host packs hidden/weights into the exact SBUF layouts so each tensor needs
  a handful of large DMAs; startup is finely interleaved so the first matmul
  fires ~2us in.
- RoPE's half-swap is a signed 64x64 permutation matmul on the PE; rope and
  V-transpose emissions for block b are deferred into block b+1's k-loop so
  the PE FIFO is never blocked by a DVE-dependent instruction.
- Attention tiles narrowed at causal/window edges (N even, >=256 to satisfy
  s3d3 fp32r ISA rules at full rate); chunk loop is software-pipelined
  (QK of chunk c+2 is emitted before PV/pd of chunk c).
- Output projection for q-block qb-1 is emitted interleaved after each head
  of q-block qb, so stage-4 matmuls fill the PE while DVE finishes the
  softmax normalization of qb.
"""

import numpy as np
import ml_dtypes
import concourse.bass as bass
import concourse.mybir as mybir
import concourse.tile as tile
from concourse.bass_utils import run_bass_kernel_spmd

F32 = mybir.dt.float32
F32R = mybir.dt.float32r
BF16 = mybir.dt.bfloat16
ACTF = mybir.ActivationFunctionType
ALU = mybir.AluOpType

B, S, D = 2, 2048, 2048
H, KVH, HD = 16, 4, 128
ROT, LORA, WINDOW = 64, 512, 1024
ROPE_BASE = 10000.0
SCALE = HD ** -0.5

HPC = H // KVH          # 4 q heads per core
SB = 512                # free-dim block for matmuls
NSB = S // SB           # 4 seq blocks
KT = D // 128           # 16 contraction tiles over D
ST = S // 128           # 16 seq 128-chunks
N_CORES = 8
PIPE = 1                # attention chunk software-pipeline depth


def _split_multiwaits(nc):
    """This image's walrus accepts only one embedded SyncWait per instruction;
    split Tile's multi-wait sync_infos into standalone event-semaphore waits."""
    n = 0
    for func in nc.m.functions:
        for bb in func.blocks:
            insts = list(bb.instructions)
            out = []
            changed = False
            for inst in insts:
                si = inst.sync_info
                if si is not None and si.on_wait and len(si.on_wait) > 1:
                    waits = list(si.on_wait)
                    for w in waits[:-1]:
                        ev = mybir.InstEventSemaphore(
                            name=f"{inst.name}_wsplit_{n}", ins=[], outs=[]
                        )
                        ev.engine = inst.engine
                        ev.sync_info = mybir.SyncInfo(on_wait=[w], on_update=[])
                        out.append(ev)
                        n += 1
                    inst.sync_info = mybir.SyncInfo(
                        on_wait=[waits[-1]], on_update=list(si.on_update or [])
                    )
                    changed = True
                out.append(inst)
            if changed:
                bb.instructions = out
    return n


def build_nc():
    nc = bass.Bass()
    # host-packed layouts: hid col = blk*8192 + k*512 + c; weff col = k*512+c;
    # wkv col = k*256+c; wo col = h*2048+c
    hid = nc.dram_tensor("hid", [128, NSB * KT * SB], BF16, kind="ExternalInput")
    weff = nc.dram_tensor("weff", [128, KT * 512], BF16, kind="ExternalInput")
    wkv = nc.dram_tensor("wkv", [128, KT * 256], BF16, kind="ExternalInput")
    wo = nc.dram_tensor("wo", [128, HPC * D], BF16, kind="ExternalInput")
    rcs = nc.dram_tensor("rcs", [128, S], F32R, kind="ExternalInput")
    out = nc.dram_tensor("out", [S, D], F32, kind="ExternalOutput")

    with tile.TileContext(nc) as tc:
        with (
            tc.tile_pool(name="cst", bufs=1) as cst,
            tc.tile_pool(name="big", bufs=1) as big,
        ):
            # ---- small constants (engine-built, no DMA) ----
            onesf = cst.tile([128, 128], F32, tag="onesf")
            nc.vector.memset(onesf[:], 1.0)
            ones = cst.tile([128, 128], F32R, tag="ones")
            nc.vector.tensor_copy(ones[:], onesf[:])
            identf = cst.tile([128, 128], F32, tag="identf")
            nc.gpsimd.affine_select(
                out=identf[:], in_=onesf[:], pattern=[[1, 128]],
                compare_op=ALU.is_equal, fill=0.0, base=0, channel_multiplier=-1,
            )
            ident = cst.tile([128, 128], F32R, tag="ident")
            nc.vector.tensor_copy(ident[:], identf[:])
            # signed rope permutation P: P[32+i, i] = -1, P[j, 32+j] = +1
            negf = cst.tile([64, 32], F32, tag="negf")
            nc.vector.memset(negf[:], -1.0)
            posf = cst.tile([64, 32], F32, tag="posf")
            nc.vector.memset(posf[:], 1.0)
            permf = cst.tile([64, 64], F32, tag="permf")
            nc.gpsimd.affine_select(
                out=permf[:, 0:32], in_=negf[:], pattern=[[-1, 32]],
                compare_op=ALU.is_equal, fill=0.0, base=-32, channel_multiplier=1,
            )
            nc.gpsimd.affine_select(
                out=permf[:, 32:64], in_=posf[:], pattern=[[-1, 32]],
                compare_op=ALU.is_equal, fill=0.0, base=0, channel_multiplier=1,
            )
            perm = cst.tile([64, 64], F32R, tag="perm")
            nc.vector.tensor_copy(perm[:], permf[:])

            # ---- persistent activations ----
            qT = big.tile([128, HPC * S], F32R, tag="qT")    # per-head Q^T [hd, s]
            kT = big.tile([128, S], F32R, tag="kT")
            vnat = big.tile([128, S], F32R, tag="vnat")      # V rows, chunk t at cols t*128
            wo_sb = big.tile([128, HPC * D], BF16, tag="wo_sb")

            with (
                tc.tile_pool(name="wp", bufs=1) as wp,
                tc.tile_pool(name="hp", bufs=5) as hp,
                tc.tile_pool(name="vt", bufs=2) as vt,
                tc.tile_pool(name="rp", bufs=2) as rp,
                tc.tile_pool(name="psA", bufs=1, space="PSUM") as psA,
                tc.tile_pool(name="psT", bufs=1, space="PSUM") as psT,
                tc.tile_pool(name="psP", bufs=1, space="PSUM") as psP,
            ):
                weff_sb = wp.tile([128, KT * 512], BF16, tag="weff_sb")
                wkv_sb = wp.tile([128, KT * 256], BF16, tag="wkv_sb")
                ropeCC = wp.tile([64, S], F32R, tag="ropeCC")
                ropeSS = wp.tile([64, S], F32R, tag="ropeSS")

                def dma_w(g0, g1):
                    nc.sync.dma_start(
                        out=weff_sb[:, g0 * 512:g1 * 512],
                        in_=weff[:, g0 * 512:g1 * 512],
                    )
                    nc.sync.dma_start(
                        out=wkv_sb[:, g0 * 256:g1 * 256],
                        in_=wkv[:, g0 * 256:g1 * 256],
                    )

                def dma_ht(sb_i, qd):
                    t = hp.tile([128, 2048], BF16, tag="ht", name=f"ht_{sb_i}_{qd}")
                    nc.sync.dma_start(
                        out=t[:],
                        in_=hid[:, sb_i * 8192 + qd * 2048:
                                sb_i * 8192 + (qd + 1) * 2048],
                    )
                    return t

                # startup interleave: weight k-groups racing block-0 hidden
                h0t = []
                dma_w(0, 1)
                t00 = hp.tile([128, 2048], BF16, tag="ht", name="ht_0_0")
                nc.sync.dma_start(out=t00[:, 0:512], in_=hid[:, 0:512])
                h0t.append(t00)
                dma_w(1, 4)
                nc.sync.dma_start(out=t00[:, 512:2048], in_=hid[:, 512:2048])
                h0t.append(dma_ht(0, 1))
                dma_w(4, 8)
                h0t.append(dma_ht(0, 2))
                dma_w(8, 12)
                h0t.append(dma_ht(0, 3))
                dma_w(12, 16)
                nc.sync.dma_start(out=ropeCC[:], in_=rcs[0:64, :])
                nc.sync.dma_start(out=ropeSS[:], in_=rcs[64:128, :])

                def rope_emit_dma(dst, csl, rsl):
                    # PE-free variant for the last block's flush: the swap
                    # goes through two small SBUF->SBUF DMAs instead of the
                    # permutation matmul, keeping the PE FIFO clear at the
                    # stage-1 -> stage-3 transition.
                    swp = rp.tile([64, SB], F32R, tag="swp")
                    nc.sync.dma_start(out=swp[0:32, :], in_=dst[32:64, csl])
                    nc.sync.dma_start(out=swp[32:64, :], in_=dst[0:32, csl])
                    csb = rp.tile([64, SB], F32R, tag="csb2")
                    nc.vector.tensor_mul(csb[:], dst[0:64, csl], ropeCC[:, rsl])
                    tsin = rp.tile([64, SB], F32R, tag="tsin2")
                    nc.vector.tensor_mul(tsin[:], swp[:], ropeSS[:, rsl])
                    nc.vector.tensor_sub(dst[0:32, csl], csb[0:32, :], tsin[0:32, :])
                    nc.vector.tensor_add(dst[32:64, csl], csb[32:64, :], tsin[32:64, :])

                def rope_emit(dst, csl, rsl):
                    # dst rows 0:64 hold [x1; x2]; out = x*cos + P^T(x*sin)
                    tsin = rp.tile([64, SB], F32R, tag="tsin")
                    nc.vector.tensor_mul(tsin[:], dst[0:64, csl], ropeSS[:, rsl])
                    csb = rp.tile([64, SB], F32R, tag="csb")
                    nc.vector.tensor_mul(csb[:], dst[0:64, csl], ropeCC[:, rsl])
                    pP = psP.tile([64, SB], F32, tag="pP")
                    nc.tensor.matmul(pP[:], perm[:], tsin[:], start=True, stop=True)
                    nc.vector.tensor_add(dst[0:64, csl], csb[:], pP[:])

                # ---- stage 1: q^T (folded), k^T, v -> vnat ----
                pending = []   # deferred rope/transpose emissions from prev block
                for sb_i in range(NSB):
                    sl = slice(sb_i * SB, (sb_i + 1) * SB)
                    if sb_i > 0:
                        hts = [dma_ht(sb_i, qd) for qd in range(4)]
                    else:
                        hts = h0t
                    pq = [
                        psA.tile([128, SB], F32, tag=f"pq{m}", name=f"pq{m}_{sb_i}")
                        for m in range(HPC)
                    ]
                    pk = psA.tile([128, SB], F32, tag="pk")
                    pv = psA.tile([128, SB], F32, tag="pv")
                    for k in range(KT):
                        hsl = hts[k // 4][:, (k % 4) * 512:(k % 4 + 1) * 512]
                        st, sp = (k == 0), (k == KT - 1)
                        for m in range(HPC):
                            nc.tensor.matmul(
                                pq[m][:],
                                weff_sb[:, k * 512 + m * 128: k * 512 + (m + 1) * 128],
                                hsl, start=st, stop=sp,
                            )
                        nc.tensor.matmul(
                            pk[:], wkv_sb[:, k * 256: k * 256 + 128], hsl,
                            start=st, stop=sp,
                        )
                        nc.tensor.matmul(
                            pv[:], wkv_sb[:, k * 256 + 128: k * 256 + 256], hsl,
                            start=st, stop=sp,
                        )
                        if k in (2, 4, 6, 8, 10, 12) and pending:
                            pending.pop(0)()
                    # evacuate this block; defer rope/transposes into next block
                    newpend = []
                    last = sb_i == NSB - 1
                    for m in range(HPC):
                        qsl = slice(m * S + sb_i * SB, m * S + (sb_i + 1) * SB)
                        nc.scalar.copy(qT[:, qsl], pq[m][:])
                        fn = rope_emit_dma if last else rope_emit
                        newpend.append(
                            lambda qsl=qsl, sl=sl, fn=fn: fn(qT, qsl, sl)
                        )
                    nc.scalar.copy(kT[:, sl], pk[:])
                    fn = rope_emit_dma if last else rope_emit
                    newpend.append(lambda sl=sl, fn=fn: fn(kT, sl, sl))
                    vtmp = vt.tile([128, SB], F32R, tag="vtmp")
                    nc.scalar.copy(vtmp[:], pv[:])

                    def emit_transposes(sb_i=sb_i, vtmp=vtmp):
                        for tt in range(4):
                            tp = psT.tile([128, 128], F32R, tag="tp")
                            nc.tensor.transpose(
                                tp[:], vtmp[:, tt * 128:(tt + 1) * 128], ident[:]
                            )
                            nc.vector.tensor_copy(
                                vnat[:, (sb_i * 4 + tt) * 128:
                                     (sb_i * 4 + tt + 1) * 128],
                                tp[:],
                            )
                    newpend.append(emit_transposes)
                    for fn in pending:   # anything not yet flushed
                        fn()
                    pending = newpend
                for fn in pending:
                    fn()
                pending = []

            # ---- Wo prefetch (lands well before stage 4 needs it) ----
            nc.sync.dma_start(out=wo_sb[:], in_=wo[:, :])

            # ---- stage 3+4: attention with interleaved output projection ----
            with (
                tc.tile_pool(name="at", bufs=2) as at,
                tc.tile_pool(name="ex", bufs=5) as ex,
                tc.tile_pool(name="rc", bufs=2) as rc,
                tc.tile_pool(name="ob", bufs=2) as ob,
                tc.tile_pool(name="psL", bufs=2, space="PSUM") as psL,
                tc.tile_pool(name="psO", bufs=2, space="PSUM") as psO,
                tc.tile_pool(name="psD", bufs=2, space="PSUM") as psD,
                tc.tile_pool(name="psW", bufs=2, space="PSUM") as psW,
            ):
                def stage4_chunk(qbx, tl, atile):
                    # out[q-chunk t, :] partial = attn(:, t-cols) @ Wo
                    t = qbx * 4 + tl
                    obuf = ob.tile([128, D], F32, tag="obuf")
                    for n4 in range(4):
                        pw = psW.tile([128, SB], F32, tag="pw")
                        for hh in range(HPC):
                            nc.tensor.matmul(
                                pw[:],
                                atile[:, hh * SB + tl * 128:
                                      hh * SB + (tl + 1) * 128],
                                wo_sb[:, hh * D + n4 * SB: hh * D + (n4 + 1) * SB],
                                start=(hh == 0), stop=(hh == HPC - 1),
                            )
                        o0 = n4 * SB
                        nc.scalar.copy(obuf[:, o0:o0 + SB], pw[:])
                        if n4 == 1:
                            nc.sync.dma_start(
                                out=out[t * 128:(t + 1) * 128, 0:2 * SB],
                                in_=obuf[:, 0:2 * SB],
                            )
                        elif n4 == 3:
                            nc.sync.dma_start(
                                out=out[t * 128:(t + 1) * 128, 2 * SB:D],
                                in_=obuf[:, 2 * SB:D],
                            )

                prev_attnT = None
                for qb in range(NSB):
                    q0 = qb * SB
                    kt_lo = max(0, q0 - WINDOW + 1) // 128
                    kt_hi = q0 // 128 + 3
                    attnT = at.tile([128, HPC * SB], BF16, tag="attnT",
                                    name=f"attnT_{qb}")
                    for h in range(HPC):
                        po = psO.tile([128, SB], F32, tag="po")
                        pd = psD.tile([128, SB], F32, tag="pd")
                        inflight = []

                        def flush_one():
                            kt, lo, hi, e = inflight.pop(0)
                            ksl = slice(kt * 128, (kt + 1) * 128)
                            st, sp = (kt == kt_lo), (kt == kt_hi)
                            nc.tensor.matmul(
                                po[:, lo:hi], vnat[:, ksl], e[:, lo:hi],
                                start=st, stop=sp,
                            )
                            nc.tensor.matmul(
                                pd[:, lo:hi], ones[:], e[:, lo:hi],
                                start=st, stop=sp,
                            )

                        for kt in range(kt_lo, kt_hi + 1):
                            dp = kt * 128 - q0
                            # valid cols [lo, hi): N even and >= 256 for the
                            # s3d3 fp32r full-rate ISA rules
                            lo = min(max(0, dp), SB - 256)
                            hi_i = dp + WINDOW + 127
                            hi = max(min(SB, hi_i + (hi_i & 1)), 256)
                            n = hi - lo
                            ksl = slice(kt * 128, (kt + 1) * 128)
                            pl = psL.tile([128, SB], F32, tag="pl")
                            nc.tensor.matmul(
                                pl[:, lo:hi], kT[:, ksl],
                                qT[:, h * S + q0 + lo: h * S + q0 + hi],
                                start=True, stop=True,
                            )
                            e = ex.tile([128, SB], F32R, tag="e")
                            nc.scalar.activation(
                                e[:, lo:hi], pl[:, lo:hi], ACTF.Exp, scale=SCALE
                            )
                            if dp >= 0:
                                # causal: keep j - i - dp >= 0 (local f = j - lo)
                                nc.gpsimd.affine_select(
                                    out=e[:, lo:hi], in_=e[:, lo:hi],
                                    pattern=[[1, n]], compare_op=ALU.is_ge,
                                    fill=0.0, base=lo - dp, channel_multiplier=-1,
                                )
                            elif dp <= -(WINDOW - SB + 1):
                                # window: keep i - j + dp + W-1 >= 0 (lo == 0)
                                nc.gpsimd.affine_select(
                                    out=e[:, lo:hi], in_=e[:, lo:hi],
                                    pattern=[[-1, n]], compare_op=ALU.is_ge,
                                    fill=0.0, base=WINDOW - 1 + dp,
                                    channel_multiplier=1,
                                )
                            inflight.append((kt, lo, hi, e))
                            if len(inflight) > PIPE:
                                flush_one()
                        while inflight:
                            flush_one()
                        rec = rc.tile([128, SB], F32, tag="rec")
                        nc.vector.reciprocal(rec[:], pd[:])
                        nc.vector.tensor_mul(
                            attnT[:, h * SB:(h + 1) * SB], po[:], rec[:]
                        )
                        if prev_attnT is not None:
                            stage4_chunk(qb - 1, h, prev_attnT)
                    prev_attnT = attnT
                for tl in range(4):
                    stage4_chunk(NSB - 1, tl, prev_attnT)
    _split_multiwaits(nc)
    return nc


_NC = None


def _get_nc():
    global _NC
    if _NC is None:
        _NC = build_nc()
    return _NC


def _make_in_maps(hidden, position_ids, Wqa, Wqb, Wk, Wv, Wo):
    hidden = np.asarray(hidden, dtype=np.float32)
    position_ids = np.asarray(position_ids)
    Wqa = np.asarray(Wqa, dtype=np.float32)
    Wqb = np.asarray(Wqb, dtype=np.float32)
    Wk = np.asarray(Wk, dtype=np.float32)
    Wv = np.asarray(Wv, dtype=np.float32)
    Wo = np.asarray(Wo, dtype=np.float32)
    weff_full = Wqa @ Wqb  # [D, H*HD]; exact assoc. fold of the LoRA Q proj

    inv_freq = 1.0 / (ROPE_BASE ** (np.arange(0, ROT, 2, dtype=np.float32) / ROT))
    in_maps = []
    for c in range(N_CORES):
        b, g = c // KVH, c % KVH
        pos = position_ids[b].astype(np.float32)
        freqs = pos[:, None] * inv_freq[None, :]        # [S, 32]
        cosT = np.cos(freqs).T.astype(np.float32)       # [32, S]
        sinT = np.sin(freqs).T.astype(np.float32)
        rcs = np.concatenate([cosT, cosT, sinT, sinT], axis=0)  # [128, S]
        hsb = (hidden[b].T.reshape(KT, 128, NSB, SB)
               .transpose(1, 2, 0, 3).reshape(128, NSB * KT * SB))
        weff = (weff_full[:, g * HPC * HD:(g + 1) * HPC * HD]
                .reshape(KT, 128, 512).transpose(1, 0, 2).reshape(128, KT * 512))
        wkv = np.concatenate(
            [Wk[:, g * HD:(g + 1) * HD], Wv[:, g * HD:(g + 1) * HD]], axis=1
        ).reshape(KT, 128, 256).transpose(1, 0, 2).reshape(128, KT * 256)
        wog = (Wo[g * HPC * HD:(g + 1) * HPC * HD, :]
               .reshape(HPC, 128, D).transpose(1, 0, 2).reshape(128, HPC * D))
        in_maps.append({
            "hid": np.ascontiguousarray(hsb.astype(ml_dtypes.bfloat16)),
            "weff": np.ascontiguousarray(weff.astype(ml_dtypes.bfloat16)),
            "wkv": np.ascontiguousarray(wkv.astype(ml_dtypes.bfloat16)),
            "wo": np.ascontiguousarray(wog.astype(ml_dtypes.bfloat16)),
            "rcs": np.ascontiguousarray(rcs),
        })
    return in_maps


def _run(inputs, trace=False):
    nc = _get_nc()
    in_maps = _make_in_maps(**inputs)
    res = run_bass_kernel_spmd(nc, in_maps, list(range(N_CORES)), trace=trace)
    out = np.zeros((B, S, D), dtype=np.float32)
    for c in range(N_CORES):
        out[c // KVH] += res.results[c]["out"]
    return out, res


def kernel(**inputs) -> np.ndarray:
    return _run(inputs, trace=False)[0]
